# revision 25
# baseline (speedup 1.0000x reference)
"""BERT-base forward on 8 Trainium2 NeuronCores, data-parallel over batch.

Each core runs the full 12-layer model on one batch element (512 tokens).
All matmul operands are bf16 (weights pre-cast on host, activations cast at
PSUM eviction); the residual/LN stream stays f32.  PE weight loads serialize
with matmuls on TRN2, so bf16 halves both HBM traffic and LDWEIGHTS time.

Layouts per core (SBUF tiles are [128 partitions, free]):
  token-major  y/ffnout (f32) and x_bf/y_bf (bf16): [128 tok, 4*768]
  hidden-major xT/QT/KT/attnT/yT (bf16): [128 hid, 6*512]
  V65 token-major bf16: [128 tok, 4*(12*65)] — 64 value dims + a ones
  column per head, so the attention-value matmul also produces the softmax
  denominator as output partition 64 (no separate ones-matmul pass).
  h1T hidden-major [128 f, 24*512] bf16.

Attention (per head pair c: heads 2c at partitions 0:64, 2c+1 at 64:128):
  S^T[k,q] = matmul(lhsT=KT[d,k-tile], rhs=QT[d,q]) row-packed pairs
  expS = Exp(S^T/8 + maskbias_k) -> bf16  (|scores/8| < 3, no max needed)
  av[0:65] = matmul(lhsT=V65, rhs=expS) summed over k chunks; row 64 = denom
  rd = approx 1/denom (DVE) -> broadcast to 64 partitions via a tiny matmul
  attnT = av[0:64] * rd_bc  (+bv) -> bf16

Token-major<->hidden-major transposes run on the DMA xbar
(dma_start_transpose, 2-byte dtype), not the PE.

LayerNorm: row sums accumulate for free during the residual-add eviction
(tensor_tensor_reduce); sum-of-squares via ACT Square+accum; rstd =
exp(-0.5*ln(var+eps)) so Exp/Ln/Square share one activation table and the
only table switches per layer are Gelu<->Exp, both preloaded off the
critical path.

Work that is provably a no-op for the given inputs (zero biases, unit
gammas, zero betas, all-ones mask) is skipped at build time; the general
path stays available and is selected per-input on the host.
"""
import os
import numpy as np
import ml_dtypes
from contextlib import ExitStack

import concourse.bass as bass
import concourse.tile as tile
from concourse import bacc, mybir
from concourse import bass_utils

f32 = mybir.dt.float32
f32r = mybir.dt.float32r
bf16 = mybir.dt.bfloat16
i32 = mybir.dt.int32
AF = mybir.ActivationFunctionType
OP = mybir.AluOpType
AX = mybir.AxisListType

V, H, L, NH, I, P, B, S = 30000, 768, 12, 12, 3072, 512, 8, 512
D = H // NH          # 64
HC = H // 128        # 6 hidden chunks
FC = I // 128        # 24 ffn chunks
TT = S // 128        # 4 token tiles
HD = D + 1           # 65: value dims + denominator ones column
LN_EPS = 1e-3

LAST_EXEC_TIME_NS = None


def _act_preload(nc, pools, func):
    """Touch `func` on a dummy so its table load lands off the critical path."""
    vec = pools["vec"]
    j = vec.tile([128, 1], f32, tag="pre", name="act_pre")
    nc.vector.memset(j[:], 1.0)
    nc.scalar.activation(j[:], j[:], func)


def _ln_bcast(nc, pools, g_row, b_row, affine):
    if not affine:
        return None, None
    gb = pools["gb"]
    g_bc = gb.tile([128, H], f32, tag="gb", name="g_bc")
    nc.sync.dma_start(g_bc[:], g_row[None, :].partition_broadcast(128))
    b_bc = gb.tile([128, H], f32, tag="gb", name="b_bc")
    nc.sync.dma_start(b_bc[:], b_row[None, :].partition_broadcast(128))
    return g_bc, b_bc


def _ln_tile(nc, pools, z, zout, tt, s_col, ssq_col, g_bc, b_bc):
    """LN tile tt of z (f32, sums already in s_col) -> zout tile (bf16/f32).

    var = E[x^2] - mu^2; rstd = exp(-0.5*ln(var+eps)) so no Sqrt table is
    needed (Ln/Exp/Square share a table with the attention Exp).
    """
    vec = pools["vec"]
    sl = slice(tt * H, (tt + 1) * H)
    nc.scalar.activation(pools["sq_scratch"][:], z[:, sl], AF.Square,
                         accum_out=ssq_col)
    b2 = vec.tile([128, 1], f32, tag="v", name=f"ln_b2_{tt}")
    nc.vector.tensor_scalar(out=b2[:], in0=s_col, scalar1=s_col,
                            scalar2=float(-1.0 / (H * H)), op0=OP.mult,
                            op1=OP.mult)
    nc.vector.tensor_scalar(out=b2[:], in0=b2[:], scalar1=float(LN_EPS),
                            scalar2=None, op0=OP.add)
    lnv = vec.tile([128, 1], f32, tag="v", name=f"ln_lnv_{tt}")
    nc.scalar.activation(lnv[:], ssq_col, AF.Ln, bias=b2[:], scale=1.0 / H)
    rstd = vec.tile([128, 1], f32, tag="v", name=f"ln_rstd_{tt}")
    nc.scalar.activation(rstd[:], lnv[:], AF.Exp, scale=-0.5)
    mr = vec.tile([128, 1], f32, tag="v", name=f"ln_mr_{tt}")
    nc.vector.tensor_scalar(out=mr[:], in0=s_col, scalar1=rstd[:],
                            scalar2=float(-1.0 / H), op0=OP.mult, op1=OP.mult)
    nc.vector.tensor_scalar(out=zout[:, sl], in0=z[:, sl], scalar1=rstd[:],
                            scalar2=mr[:], op0=OP.mult, op1=OP.add)
    if g_bc is not None:
        nc.vector.tensor_tensor(out=zout[:, sl], in0=zout[:, sl], in1=g_bc[:],
                                op=OP.mult)
    if b_bc is not None:
        nc.vector.tensor_tensor(out=zout[:, sl], in0=zout[:, sl], in1=b_bc[:],
                                op=OP.add)


TP_MODE = os.environ.get("BERT_TP", "dma")      # 'dma' xbar | 'pe' tensor engine
BCAST_MODE = os.environ.get("BERT_BCAST", "gpsimd")  # 'gpsimd' | 'pe'
# CoreSim lacks Gelu; BERT_SIMACT=1 swaps in Tanh (same dataflow) for sim runs
GELU_AF = AF.Tanh if os.environ.get("BERT_SIMACT") else AF.Gelu


def _dma_transpose_tile(nc, src_bf, dst, tt, pools):
    """src_bf[:, tt*768:(tt+1)*768] (tok-major bf16) -> dst hidden-major cols."""
    if TP_MODE == "dma":
        out_view = dst[:, tt * H:(tt + 1) * H].rearrange("p (c t) -> p c t", c=HC)
        nc.sync.dma_start_transpose(out_view, src_bf[:, tt * H:(tt + 1) * H])
    else:
        psT, ident = pools["psT"], pools["ident"]
        for c in range(HC):
            tp = psT.tile([128, 128], bf16, tag="tp", name="tp")
            nc.tensor.transpose(tp[:],
                                src_bf[:, tt * H + c * 128: tt * H + c * 128 + 128],
                                ident[:])
            nc.vector.tensor_copy(
                dst[:, tt * H + c * 128: tt * H + c * 128 + 128], tp[:])


def build(n_layers=L, flags=None):
    fl = flags or {}
    qk_bias = fl.get("qk_bias", True)
    v_bias = fl.get("v_bias", True)
    o_bias = fl.get("o_bias", True)
    i_bias = fl.get("i_bias", True)
    d_bias = fl.get("d_bias", True)
    ln1_aff = fl.get("ln1_aff", True)
    ln2_aff = fl.get("ln2_aff", True)
    emb_aff = fl.get("emb_aff", True)
    use_mask = fl.get("use_mask", True)
    use_type = fl.get("use_type", True)

    nc = bacc.Bacc("TRN2", target_bir_lowering=False, debug=False, num_devices=8)

    dt_in = lambda n, s, d: nc.dram_tensor(n, s, d, kind="ExternalInput").ap()
    ids_d = dt_in("ids", [S], i32)
    tti_d = dt_in("tti", [S], i32)
    mb_d = dt_in("mb", [S], f32)
    tok_d = dt_in("tok_emb", [V, H], f32)
    pos_d = dt_in("pos_emb", [S, H], f32)
    typ_d = dt_in("type_emb", [2, H], f32)
    eg_d = dt_in("emb_g", [H], f32)
    eb_d = dt_in("emb_b", [H], f32)
    wq_d = dt_in("WqS", [L, HC, 128, HC, 128], bf16)
    wk_d = dt_in("WkS", [L, HC, 128, HC, 128], bf16)
    wv_d = dt_in("WvS", [L, 2, 128, HC, 384], bf16)
    wo_d = dt_in("WoS", [L, 2, 128, HC, 384], bf16)
    wi_d = dt_in("WiS", [L, FC, 128, HC, 128], bf16)
    wd_d = dt_in("WdB", [L, FC // 4, 128, 4, H], bf16)
    bq_d = dt_in("bq", [L, H], f32)
    bk_d = dt_in("bk", [L, H], f32)
    bv_d = dt_in("bv", [L, H], f32)
    bo_d = dt_in("bo", [L, H], f32r)
    bi_d = dt_in("bi", [L, I], f32)
    bd_d = dt_in("bd", [L, H], f32r)
    g1_d = dt_in("ln1_g", [L, H], f32)
    b1_d = dt_in("ln1_b", [L, H], f32)
    g2_d = dt_in("ln2_g", [L, H], f32)
    b2_d = dt_in("ln2_b", [L, H], f32)
    ones_d = dt_in("ones", [128, 128], f32r)
    ident_d = dt_in("ident", [128, 128], bf16)
    out_d = nc.dram_tensor("out", [S, H], f32, kind="ExternalOutput").ap()
    DBG = bool(os.environ.get("BERT_DBG"))
    dbg = {}
    if DBG:
        for nm, w, dt in (("x_bf", TT * H, bf16), ("xT", TT * H, bf16),
                          ("QT", TT * H, bf16), ("KT", TT * H, bf16),
                          ("V65", TT * NH * HD, bf16), ("attnT", TT * H, bf16),
                          ("y", TT * H, f32), ("y_bf", TT * H, bf16),
                          ("yT", TT * H, bf16), ("h1T", FC * S, bf16),
                          ("ffnout", TT * H, f32), ("es00", S, bf16), ("es01", S, bf16), ("es02", S, bf16), ("es03", S, bf16),
                          ("av0", S, f32), ("rd0", S, f32), ("bcs0", S, f32)):
            dbg[nm] = nc.dram_tensor(f"dbg_{nm}", [128, w], dt,
                                     kind="ExternalOutput").ap()

    def dump(nm, t):
        if DBG:
            nc.sync.dma_start(dbg[nm][:], t[:])

    with tile.TileContext(nc) as tc, ExitStack() as ctx:
        tb = ctx.enter_context(tc.tile_pool(name="tb", bufs=5))       # bf16 hidden-major
        resf = ctx.enter_context(tc.tile_pool(name="resf", bufs=2))   # f32 residual
        resbf = ctx.enter_context(tc.tile_pool(name="resbf", bufs=2))  # bf16 post-LN
        v65p = ctx.enter_context(tc.tile_pool(name="v65p", bufs=1))
        h1p = ctx.enter_context(tc.tile_pool(name="h1p", bufs=1))
        wbig = ctx.enter_context(tc.tile_pool(name="wbig", bufs=4))
        wsmall = ctx.enter_context(tc.tile_pool(name="wsmall", bufs=6))
        wdp = ctx.enter_context(tc.tile_pool(name="wdp", bufs=3))
        gb = ctx.enter_context(tc.tile_pool(name="gb", bufs=2))
        exps_p = ctx.enter_context(tc.tile_pool(name="exps_p", bufs=12))
        rd_p = ctx.enter_context(tc.tile_pool(name="rd_p", bufs=2))
        scratch = ctx.enter_context(tc.tile_pool(name="scratch", bufs=1))
        vec = ctx.enter_context(tc.tile_pool(name="vec", bufs=24))
        stats = ctx.enter_context(tc.tile_pool(name="stats", bufs=3))
        brow_p = ctx.enter_context(tc.tile_pool(name="brow_p", bufs=1))
        const = ctx.enter_context(tc.tile_pool(name="const", bufs=1))
        psV_bufs = 1 if TP_MODE == "pe" else 2
        psA_bufs = (8 - psV_bufs - (2 if TP_MODE == "pe" else 0)
                    - (1 if BCAST_MODE == "pe" else 0))
        psA = ctx.enter_context(tc.tile_pool(name="psA", bufs=psA_bufs, space="PSUM"))
        psV = ctx.enter_context(tc.tile_pool(name="psV", bufs=psV_bufs, space="PSUM"))
        if TP_MODE == "pe":
            psT = ctx.enter_context(tc.tile_pool(name="psT", bufs=2, space="PSUM"))
        if BCAST_MODE == "pe":
            psB = ctx.enter_context(tc.tile_pool(name="psB", bufs=1, space="PSUM"))
        pools = dict(gb=gb, vec=vec)
        pools["sq_scratch"] = scratch.tile([128, H], f32, tag="sq", name="sq_scratch")

        # constants
        ones_sb = const.tile([128, 128], f32r, tag="ones", name="ones_sb")
        nc.sync.dma_start(ones_sb[:], ones_d[:])
        if TP_MODE == "pe":
            ident = const.tile([128, 128], bf16, tag="ident", name="ident")
            nc.sync.dma_start(ident[:], ident_d[:])
            pools["psT"] = psT
            pools["ident"] = ident
        if BCAST_MODE == "pe":
            ones_bf = const.tile([1, 64], bf16, tag="ones_bf", name="ones_bf")
            nc.vector.memset(ones_bf[:], 1.0)
        ids_sb = const.tile([128, TT], i32, tag="ids", name="ids_sb")
        nc.sync.dma_start(ids_sb[:], ids_d.rearrange("(t p) -> p t", p=128))
        if use_type:
            tti_sb = const.tile([128, TT], i32, tag="tti", name="tti_sb")
            nc.sync.dma_start(tti_sb[:], tti_d.rearrange("(t p) -> p t", p=128))
        if use_mask:
            mb_sb = const.tile([128, TT], f32, tag="mb", name="mb_sb")
            nc.sync.dma_start(mb_sb[:], mb_d.rearrange("(t p) -> p t", p=128))

        _act_preload(nc, pools, AF.Exp)  # expln table resident from the start

        # ---- embedding -> x (f32) -> LN -> x_bf (bf16) ----
        x_emb = resf.tile([128, TT * H], f32, tag="res", name="x_emb")
        eg_bc, eb_bc = _ln_bcast(nc, pools, eg_d, eb_d, emb_aff)
        s_emb = stats.tile([128, TT], f32, tag="s", name="s_emb")
        ssq_emb = stats.tile([128, TT], f32, tag="ssq", name="ssq_emb")
        x_bf = resbf.tile([128, TT * H], bf16, tag="rbf", name="x_bf_emb")
        for tt in range(TT):
            sl = slice(tt * H, (tt + 1) * H)
            nc.gpsimd.indirect_dma_start(
                out=x_emb[:, sl], out_offset=None, in_=tok_d[:],
                in_offset=bass.IndirectOffsetOnAxis(ap=ids_sb[:, tt:tt + 1], axis=0))
            if use_type:
                tmp_t = gb.tile([128, H], f32, tag="gb", name="emb_tmp")
                nc.gpsimd.indirect_dma_start(
                    out=tmp_t[:], out_offset=None, in_=typ_d[:],
                    in_offset=bass.IndirectOffsetOnAxis(ap=tti_sb[:, tt:tt + 1], axis=0))
                nc.vector.tensor_tensor(out=x_emb[:, sl], in0=x_emb[:, sl],
                                        in1=tmp_t[:], op=OP.add)
            tmp_p = gb.tile([128, H], f32, tag="gb", name="emb_pos")
            nc.sync.dma_start(tmp_p[:], pos_d[tt * 128:(tt + 1) * 128, :])
            nc.vector.tensor_tensor(out=x_emb[:, sl], in0=x_emb[:, sl],
                                    in1=tmp_p[:], op=OP.add)
            nc.vector.reduce_sum(out=s_emb[:, tt:tt + 1], in_=x_emb[:, sl],
                                 axis=AX.X)
            _ln_tile(nc, pools, x_emb, x_bf, tt, s_emb[:, tt:tt + 1],
                     ssq_emb[:, tt:tt + 1], eg_bc, eb_bc)

        # ---- layers ----
        dump("x_bf", x_bf)
        for l in range(n_layers):
            last = (l == n_layers - 1)

            xT = tb.tile([128, HC * S], bf16, tag="t", name=f"xT_{l}")
            for tt in range(TT):
                _dma_transpose_tile(nc, x_bf, xT, tt, pools)
            if l == 0:
                dump("xT", xT)

            # Q^T, K^T hidden-major; token-halves so early chunks start sooner
            QT = tb.tile([128, HC * S], bf16, tag="t", name=f"QT_{l}")
            KT = tb.tile([128, HC * S], bf16, tag="t", name=f"KT_{l}")
            for dst, w_d, b_d in ((QT, wq_d, bq_d), (KT, wk_d, bk_d)):
                for j in range(HC):
                    wblk = wsmall.tile([128, HC, 128], bf16, tag="ws", name="wqk_blk")
                    nc.sync.dma_start(wblk[:], w_d[l, j])
                    pq = psA.tile([128, S], f32, tag="main", name="pq")
                    xTv = xT.rearrange("p (f c t) -> p f c t", f=TT, c=HC)
                    for half in range(2):
                        hs = slice(half * 256, half * 256 + 256)
                        for ic in range(HC):
                            nc.tensor.matmul(
                                pq[:, hs], lhsT=wblk[:, ic, :],
                                rhs=xTv[:, 2 * half:2 * half + 2, ic, :],
                                start=(ic == 0), stop=(ic == HC - 1))
                    jsl = slice(j * S, (j + 1) * S)
                    if qk_bias:
                        b_sl = vec.tile([128, 1], f32, tag="v", name="bqk_sl")
                        nc.sync.dma_start(b_sl[:], b_d[l, j * 128:(j + 1) * 128][:, None])
                        nc.scalar.activation(dst[:, jsl], pq[:], AF.Identity,
                                             bias=b_sl[:])
                    else:
                        nc.vector.tensor_copy(dst[:, jsl], pq[:])

            if l == 0:
                dump("QT", QT)
                dump("KT", KT)
            # V token-major with a ones column per head (denominator fold)
            V65t = v65p.tile([128, TT * NH * HD], bf16, tag="v65", name=f"V65_{l}")
            ones_cols = V65t.rearrange("p (f h d) -> p f h d", f=TT, h=NH)[:, :, :, D]
            nc.vector.memset(ones_cols, 1.0)
            for n in range(2):
                wvblk = wbig.tile([128, HC, 384], bf16, tag="wb", name="wv_blk")
                nc.sync.dma_start(wvblk[:], wv_d[l, n])
                for tt in range(TT):
                    pv = psA.tile([128, 384], f32, tag="main", name="pv")
                    for ic in range(HC):
                        nc.tensor.matmul(
                            pv[:], lhsT=xT[:, tt * H + ic * 128: tt * H + ic * 128 + 128],
                            rhs=wvblk[:, ic, :],
                            start=(ic == 0), stop=(ic == HC - 1))
                    # strided copy: 6 heads' 64-wide blocks into 65-wide slots
                    dst = V65t.rearrange("p (f h d) -> p f h d", f=TT, h=NH)[
                        :, tt, n * 6:(n + 1) * 6, 0:D]
                    src = pv.rearrange("p (h d) -> p h d", h=6)
                    nc.vector.tensor_copy(dst, src)

            if l == 0:
                dump("V65", V65t)
            # attention, head pairs
            attnT = tb.tile([128, HC * S], bf16, tag="t", name=f"attnT_{l}")
            v65v = V65t.rearrange("p (f h d) -> p f h d", f=TT, h=NH)
            for c in range(HC):
                es = [[None] * TT for _ in range(2)]
                for hh in range(2):
                    r0 = 64 * hh
                    for kc in range(TT):
                        sp = psA.tile([128, S], f32, tag="main", name="sp")
                        nc.tensor.matmul(
                            sp[:],
                            lhsT=KT[r0:r0 + 64, c * S + kc * 128: c * S + kc * 128 + 128],
                            rhs=QT[r0:r0 + 64, c * S:(c + 1) * S],
                            start=True, stop=True)
                        e = exps_p.tile([128, S], bf16, tag="e", name=f"e{hh}_{kc}")
                        mbias = mb_sb[:, kc:kc + 1] if use_mask else 0.0
                        nc.scalar.activation(e[:], sp[:], AF.Exp,
                                             bias=mbias, scale=0.125)
                        es[hh][kc] = e
                for hh in range(2):
                    h = 2 * c + hh
                    av = psV.tile([HD, S], f32, tag="av", name="av")
                    for kc in range(TT):
                        nc.tensor.matmul(
                            av[:], lhsT=v65v[:, kc, h, :], rhs=es[hh][kc][:],
                            start=(kc == 0), stop=(kc == TT - 1))
                    # regular-op copy first: a custom-DVE read of a PSUM
                    # accumulation group races ahead of the group's tail
                    dn = rd_p.tile([1, S], f32, tag="dn", name="dn")
                    nc.vector.tensor_copy(dn[:], av[D:HD, :])
                    rd = rd_p.tile([1, S], f32, tag="rd", name="rd")
                    nc.vector.reciprocal_approx_fast(out=rd[:], in_=dn[:])
                    if DBG and l == 0 and c == 0 and hh == 0:
                        for _kc in range(TT):
                            nc.sync.dma_start(dbg[f"es0{_kc}"][0:128, :],
                                              es[0][_kc][:])
                        avd = gb.tile([128, S], f32, tag="gb", name="avd")
                        nc.vector.tensor_copy(avd[0:HD, :], av[:])
                        nc.sync.dma_start(dbg["av0"][0:HD, :], avd[0:HD, :])
                        nc.sync.dma_start(dbg["rd0"][0:1, :], rd[:])
                    bcs = rd_p.tile([64, S], f32, tag="bcs", name="bcs")
                    if BCAST_MODE == "gpsimd":
                        nc.gpsimd.partition_broadcast(bcs[:], rd[0:1, :])
                    else:
                        rdb = rd_p.tile([1, S], bf16, tag="rdb", name="rdb")
                        nc.vector.tensor_copy(rdb[:], rd[:])
                        bcp = psB.tile([64, S], f32, tag="bc", name="bcp")
                        nc.tensor.matmul(bcp[:], lhsT=ones_bf[0:1, 0:64],
                                         rhs=rdb[:], start=True, stop=True)
                        nc.vector.tensor_copy(bcs[:], bcp[:])
                    if DBG and l == 0 and c == 0 and hh == 0:
                        nc.sync.dma_start(dbg["bcs0"][0:64, :], bcs[:])
                    dst = attnT[64 * hh:64 * hh + 64, c * S:(c + 1) * S]
                    nc.vector.tensor_tensor(out=dst, in0=av[0:D, :], in1=bcs[:],
                                            op=OP.mult)
                    if v_bias:
                        bv_sl = vec.tile([64, 1], f32, tag="bv", name="bv_sl")
                        nc.sync.dma_start(bv_sl[:], bv_d[l, h * D:(h + 1) * D][:, None])
                        nc.vector.tensor_scalar(
                            out=dst, in0=dst,
                            scalar1=bv_sl[:], scalar2=None, op0=OP.add)

            if l == 0:
                dump("attnT", attnT)
            # Wo projection (+bo) + residual -> y (f32), LN1 -> y_bf (bf16)
            y = resf.tile([128, TT * H], f32, tag="res", name=f"y_{l}")
            y_bf = resbf.tile([128, TT * H], bf16, tag="rbf", name=f"ybf_{l}")
            g1_bc, b1_bc = _ln_bcast(nc, pools, g1_d[l], b1_d[l], ln1_aff)
            s1 = stats.tile([128, TT], f32, tag="s", name=f"s1_{l}")
            ssq1 = stats.tile([128, TT], f32, tag="ssq", name=f"ssq1_{l}")
            if o_bias:
                bo_row = brow_p.tile([1, H], f32r, tag="br", name="bo_row")
                nc.sync.dma_start(bo_row[:], bo_d[l][None, :])
            woblks = []
            for n in range(2):
                wob = wbig.tile([128, HC, 384], bf16, tag="wb", name=f"wo_blk{n}")
                nc.sync.dma_start(wob[:], wo_d[l, n])
                woblks.append(wob)
            for tt in range(TT):
                for n in range(2):
                    po = psA.tile([128, 384], f32, tag="main", name="po")
                    if o_bias:
                        nc.tensor.matmul(po[:], lhsT=ones_sb[0:1, 0:128],
                                         rhs=bo_row[0:1, n * 384:(n + 1) * 384],
                                         start=True, stop=False)
                    for jc in range(HC):
                        nc.tensor.matmul(
                            po[:],
                            lhsT=attnT[:, jc * S + tt * 128: jc * S + tt * 128 + 128],
                            rhs=woblks[n][:, jc, :],
                            start=(not o_bias and jc == 0), stop=(jc == HC - 1))
                    sl = slice(tt * H + n * 384, tt * H + n * 384 + 384)
                    nc.vector.tensor_tensor(out=y[:, sl], in0=po[:, :],
                                            in1=x_bf[:, sl], op=OP.add)
                nc.vector.reduce_sum(out=s1[:, tt:tt + 1],
                                     in_=y[:, tt * H:(tt + 1) * H], axis=AX.X)
                _ln_tile(nc, pools, y, y_bf, tt, s1[:, tt:tt + 1],
                         ssq1[:, tt:tt + 1], g1_bc, b1_bc)

            if l == 0:
                dump("y", y)
                dump("y_bf", y_bf)
            # yT (DMA xbar), then preload the Gelu table while matmuls run
            yT = tb.tile([128, HC * S], bf16, tag="t", name=f"yT_{l}")
            for tt in range(TT):
                _dma_transpose_tile(nc, y_bf, yT, tt, pools)
            _act_preload(nc, pools, GELU_AF)
            if l == 0:
                dump("yT", yT)

            # FFN up: h1T = gelu(yT @ Wi + bi), hidden-major, bf16
            h1T = h1p.tile([128, FC * S], bf16, tag="h1", name=f"h1T_{l}")
            for fc in range(FC):
                wiblk = wsmall.tile([128, HC, 128], bf16, tag="ws", name="wi_blk")
                nc.sync.dma_start(wiblk[:], wi_d[l, fc])
                ph = psA.tile([128, S], f32, tag="main", name="ph")
                yTv = yT.rearrange("p (f c t) -> p f c t", f=TT, c=HC)
                for half in range(2):
                    hs = slice(half * 256, half * 256 + 256)
                    for ic in range(HC):
                        nc.tensor.matmul(
                            ph[:, hs], lhsT=wiblk[:, ic, :],
                            rhs=yTv[:, 2 * half:2 * half + 2, ic, :],
                            start=(ic == 0), stop=(ic == HC - 1))
                if i_bias:
                    bi_sl = vec.tile([128, 1], f32, tag="v", name="bi_sl")
                    nc.sync.dma_start(bi_sl[:], bi_d[l, fc * 128:(fc + 1) * 128][:, None])
                    nc.scalar.activation(h1T[:, fc * S:(fc + 1) * S], ph[:], GELU_AF,
                                         bias=bi_sl[:])
                else:
                    nc.scalar.activation(h1T[:, fc * S:(fc + 1) * S], ph[:], GELU_AF)

            if l == 0:
                dump("h1T", h1T)
            # FFN down (bf16) + bd + residual -> ffnout; waves of 4 (tt,n) pairs
            ffnout = resf.tile([128, TT * H], f32, tag="res", name=f"ffnout_{l}")
            xbf_next = (None if last else
                        resbf.tile([128, TT * H], bf16, tag="rbf", name=f"xbf_{l + 1}"))
            g2_bc, b2_bc = _ln_bcast(nc, pools, g2_d[l], b2_d[l], ln2_aff)
            s2 = stats.tile([128, TT], f32, tag="s", name=f"s2_{l}")
            ssq2 = stats.tile([128, TT], f32, tag="ssq", name=f"ssq2_{l}")
            if d_bias:
                bd_row = brow_p.tile([1, H], f32r, tag="br", name="bd_row")
                nc.sync.dma_start(bd_row[:], bd_d[l][None, :])
            for wave in range(2):
                tts = (0, 1) if wave == 0 else (2, 3)
                wave_pairs = [(tt, n) for tt in tts for n in range(2)]
                accs = {}
                for (tt, n) in wave_pairs:
                    acc = psA.tile([128, 384], f32, tag="main", name=f"acc{tt}_{n}")
                    if d_bias:
                        nc.tensor.matmul(acc[:], lhsT=ones_sb[0:1, 0:128],
                                         rhs=bd_row[0:1, n * 384:(n + 1) * 384],
                                         start=True, stop=False)
                    accs[(tt, n)] = acc
                for fp in range(FC // 4):
                    wdblk = wdp.tile([128, 4, H], bf16, tag="wd", name="wd_blk")
                    nc.sync.dma_start(wdblk[:], wd_d[l, fp])
                    for two in range(4):
                        fc = 4 * fp + two
                        for (tt, n) in wave_pairs:
                            nc.tensor.matmul(
                                accs[(tt, n)][:],
                                lhsT=h1T[:, fc * S + tt * 128: fc * S + tt * 128 + 128],
                                rhs=wdblk[:, two, n * 384:(n + 1) * 384],
                                start=(not d_bias and fc == 0), stop=(fc == FC - 1))
                if wave == 0:
                    # expln table back in residence before LN2's Ln/Exp
                    _act_preload(nc, pools, AF.Exp)
                for tt in tts:
                    for n in range(2):
                        sl = slice(tt * H + n * 384, tt * H + n * 384 + 384)
                        nc.vector.tensor_tensor(out=ffnout[:, sl],
                                                in0=accs[(tt, n)][:, :],
                                                in1=y_bf[:, sl], op=OP.add)
                    nc.vector.reduce_sum(out=s2[:, tt:tt + 1],
                                         in_=ffnout[:, tt * H:(tt + 1) * H],
                                         axis=AX.X)
                    if last:
                        x_out = pools.get("x_out")
                        if x_out is None:
                            x_out = scratch.tile([128, TT * H], f32, tag="xo",
                                                 name="x_out")
                            pools["x_out"] = x_out
                        _ln_tile(nc, pools, ffnout, x_out, tt, s2[:, tt:tt + 1],
                                 ssq2[:, tt:tt + 1], g2_bc, b2_bc)
                        nc.sync.dma_start(
                            out_d[tt * 128:(tt + 1) * 128, :],
                            x_out[:, tt * H:(tt + 1) * H])
                    else:
                        _ln_tile(nc, pools, ffnout, xbf_next, tt, s2[:, tt:tt + 1],
                                 ssq2[:, tt:tt + 1], g2_bc, b2_bc)
            if l == 0:
                dump("ffnout", ffnout)
            x_bf = xbf_next

    nc.compile()
    return nc


def _prep_inputs(inputs, b):
    f = np.float32
    bh = ml_dtypes.bfloat16
    Wq, Wk, Wv, Wo, Wi = (np.asarray(inputs[k], f) for k in ("Wq", "Wk", "Wv", "Wo", "Wi"))
    WqS = np.ascontiguousarray(
        Wq.reshape(L, HC, 128, HC, 128).transpose(0, 3, 2, 1, 4)).astype(bh)
    WkS = np.ascontiguousarray(
        Wk.reshape(L, HC, 128, HC, 128).transpose(0, 3, 2, 1, 4)).astype(bh)
    WvS = np.ascontiguousarray(
        Wv.reshape(L, HC, 128, 2, 384).transpose(0, 3, 2, 1, 4)).astype(bh)
    WoS = np.ascontiguousarray(
        Wo.reshape(L, HC, 128, 2, 384).transpose(0, 3, 2, 1, 4)).astype(bh)
    WiS = np.ascontiguousarray(
        Wi.reshape(L, HC, 128, FC, 128).transpose(0, 3, 2, 1, 4)).astype(bh)
    Wd = np.asarray(inputs["Wd"], f)
    WdB = np.ascontiguousarray(
        Wd.reshape(L, FC // 4, 4, 128, H).transpose(0, 1, 3, 2, 4)).astype(bh)
    mask = np.asarray(inputs["input_mask"], f)
    tti = np.asarray(inputs["token_type_ids"], np.int32)
    flags = dict(
        qk_bias=bool(np.any(np.asarray(inputs["bq"])) or np.any(np.asarray(inputs["bk"]))),
        v_bias=bool(np.any(np.asarray(inputs["bv"]))),
        o_bias=bool(np.any(np.asarray(inputs["bo"]))),
        i_bias=bool(np.any(np.asarray(inputs["bi"]))),
        d_bias=bool(np.any(np.asarray(inputs["bd"]))),
        ln1_aff=bool(np.any(np.asarray(inputs["ln1_g"]) != 1.0) or
                     np.any(np.asarray(inputs["ln1_b"]))),
        ln2_aff=bool(np.any(np.asarray(inputs["ln2_g"]) != 1.0) or
                     np.any(np.asarray(inputs["ln2_b"]))),
        emb_aff=bool(np.any(np.asarray(inputs["emb_ln_g"]) != 1.0) or
                     np.any(np.asarray(inputs["emb_ln_b"]))),
        use_mask=bool(np.any(mask != 1.0)),
        use_type=bool(np.any(tti != 0)),
    )
    pos_eff = np.asarray(inputs["pos_emb"], f)[:S]
    if not flags["use_type"]:
        pos_eff = pos_eff + np.asarray(inputs["type_emb"], f)[int(tti.flat[0])][None, :]
    shared = dict(
        tok_emb=np.asarray(inputs["tok_emb"], f),
        pos_emb=pos_eff,
        type_emb=np.asarray(inputs["type_emb"], f),
        emb_g=np.asarray(inputs["emb_ln_g"], f),
        emb_b=np.asarray(inputs["emb_ln_b"], f),
        WqS=WqS, WkS=WkS, WvS=WvS, WoS=WoS, WiS=WiS, WdB=WdB,
        bq=np.asarray(inputs["bq"], f), bk=np.asarray(inputs["bk"], f),
        bv=np.asarray(inputs["bv"], f), bo=np.asarray(inputs["bo"], f),
        bi=np.asarray(inputs["bi"], f), bd=np.asarray(inputs["bd"], f),
        ln1_g=np.asarray(inputs["ln1_g"], f), ln1_b=np.asarray(inputs["ln1_b"], f),
        ln2_g=np.asarray(inputs["ln2_g"], f), ln2_b=np.asarray(inputs["ln2_b"], f),
        ones=np.ones((128, 128), f),
        ident=np.eye(128, dtype=ml_dtypes.bfloat16),
    )
    in_maps = []
    ids = np.asarray(inputs["input_ids"], np.int32)
    for c in range(b):
        m = dict(shared)
        m["ids"] = np.ascontiguousarray(ids[c])
        m["tti"] = np.ascontiguousarray(tti[c])
        m["mb"] = np.ascontiguousarray((1.0 - mask[c]) * -10000.0)
        in_maps.append(m)
    return in_maps, flags


def kernel(**inputs):
    global LAST_EXEC_TIME_NS
    n_layers = int(os.environ.get("BERT_LAYERS", L))
    trace = bool(os.environ.get("BERT_TRACE"))
    in_maps, flags = _prep_inputs(inputs, B)
    nc = build(n_layers, flags)
    res = bass_utils.run_bass_kernel_spmd(
        nc, in_maps, core_ids=list(range(B)), trace=trace)
    LAST_EXEC_TIME_NS = res.exec_time_ns
    out = np.stack([res.results[c]["out"] for c in range(B)])
    return out.astype(np.float32)


# revision 26
# speedup vs baseline: 1.0700x; 1.0700x over previous
"""BERT-base forward on 8 Trainium2 NeuronCores, data-parallel over batch.

Each core runs the full 12-layer model on one batch element (512 tokens).
All matmul operands are bf16 (weights pre-cast on host, activations cast at
PSUM eviction); the residual/LN stream stays f32.  PE weight loads serialize
with matmuls on TRN2, so bf16 halves both HBM traffic and LDWEIGHTS time.

Layouts per core (SBUF tiles are [128 partitions, free]):
  token-major  y/ffnout (f32) and x_bf/y_bf (bf16): [128 tok, 4*768]
  hidden-major xT/QT/KT/attnT/yT (bf16): [128 hid, 6*512]
  V65 token-major bf16: [128 tok, 4*(12*65)] — 64 value dims + a ones
  column per head, so the attention-value matmul also produces the softmax
  denominator as output partition 64 (no separate ones-matmul pass).
  h1T hidden-major [128 f, 24*512] bf16.

Attention (per head pair c: heads 2c at partitions 0:64, 2c+1 at 64:128):
  S^T[k,q] = matmul(lhsT=KT[d,k-tile], rhs=QT[d,q]) row-packed pairs
  expS = Exp(S^T/8 + maskbias_k) -> bf16  (|scores/8| < 3, no max needed)
  av[0:65] = matmul(lhsT=V65, rhs=expS) summed over k chunks; row 64 = denom
  rd = approx 1/denom (DVE) -> broadcast to 64 partitions via a tiny matmul
  attnT = av[0:64] * rd_bc  (+bv) -> bf16

Token-major<->hidden-major transposes run on the DMA xbar
(dma_start_transpose, 2-byte dtype), not the PE.

LayerNorm: row sums accumulate for free during the residual-add eviction
(tensor_tensor_reduce); sum-of-squares via ACT Square+accum; rstd =
exp(-0.5*ln(var+eps)) so Exp/Ln/Square share one activation table and the
only table switches per layer are Gelu<->Exp, both preloaded off the
critical path.

Work that is provably a no-op for the given inputs (zero biases, unit
gammas, zero betas, all-ones mask) is skipped at build time; the general
path stays available and is selected per-input on the host.
"""
import os
import numpy as np
import ml_dtypes
from contextlib import ExitStack

import concourse.bass as bass
import concourse.tile as tile
from concourse import bacc, mybir
from concourse import bass_utils

f32 = mybir.dt.float32
f32r = mybir.dt.float32r
bf16 = mybir.dt.bfloat16
i32 = mybir.dt.int32
AF = mybir.ActivationFunctionType
OP = mybir.AluOpType
AX = mybir.AxisListType

V, H, L, NH, I, P, B, S = 30000, 768, 12, 12, 3072, 512, 8, 512
D = H // NH          # 64
HC = H // 128        # 6 hidden chunks
FC = I // 128        # 24 ffn chunks
TT = S // 128        # 4 token tiles
HD = D + 1           # 65: value dims + denominator ones column
LN_EPS = 1e-3

LAST_EXEC_TIME_NS = None


def _act_preload(nc, pools, func):
    """Touch `func` on a dummy so its table load lands off the critical path."""
    vec = pools["vec"]
    j = vec.tile([128, 1], f32, tag="pre", name="act_pre")
    nc.vector.memset(j[:], 1.0)
    nc.scalar.activation(j[:], j[:], func)


def _ln_bcast(nc, pools, g_row, b_row, affine):
    if not affine:
        return None, None
    gb = pools["gb"]
    g_bc = gb.tile([128, H], f32, tag="gb", name="g_bc")
    nc.sync.dma_start(g_bc[:], g_row[None, :].partition_broadcast(128))
    b_bc = gb.tile([128, H], f32, tag="gb", name="b_bc")
    nc.sync.dma_start(b_bc[:], b_row[None, :].partition_broadcast(128))
    return g_bc, b_bc


def _ln_tile(nc, pools, z, zout, tt, s_col, ssq_col, g_bc, b_bc):
    """LN tile tt of z (f32, sums already in s_col) -> zout tile (bf16/f32).

    var = E[x^2] - mu^2; rstd = exp(-0.5*ln(var+eps)) so no Sqrt table is
    needed (Ln/Exp/Square share a table with the attention Exp).
    """
    vec = pools["vec"]
    sl = slice(tt * H, (tt + 1) * H)
    nc.scalar.activation(pools["sq_scratch"][:], z[:, sl], AF.Square,
                         accum_out=ssq_col)
    b2 = vec.tile([128, 1], f32, tag="v", name=f"ln_b2_{tt}")
    nc.vector.tensor_scalar(out=b2[:], in0=s_col, scalar1=s_col,
                            scalar2=float(-1.0 / (H * H)), op0=OP.mult,
                            op1=OP.mult)
    nc.vector.tensor_scalar(out=b2[:], in0=b2[:], scalar1=float(LN_EPS),
                            scalar2=None, op0=OP.add)
    sd = vec.tile([128, 1], f32, tag="v", name=f"ln_sd_{tt}")
    nc.scalar.activation(sd[:], ssq_col, AF.Sqrt, bias=b2[:], scale=1.0 / H)
    rstd = vec.tile([128, 1], f32, tag="v", name=f"ln_rstd_{tt}")
    nc.vector.reciprocal(rstd[:], sd[:])
    mr = vec.tile([128, 1], f32, tag="v", name=f"ln_mr_{tt}")
    nc.vector.tensor_scalar(out=mr[:], in0=s_col, scalar1=rstd[:],
                            scalar2=float(-1.0 / H), op0=OP.mult, op1=OP.mult)
    nc.vector.tensor_scalar(out=zout[:, sl], in0=z[:, sl], scalar1=rstd[:],
                            scalar2=mr[:], op0=OP.mult, op1=OP.add)
    if g_bc is not None:
        nc.vector.tensor_tensor(out=zout[:, sl], in0=zout[:, sl], in1=g_bc[:],
                                op=OP.mult)
    if b_bc is not None:
        nc.vector.tensor_tensor(out=zout[:, sl], in0=zout[:, sl], in1=b_bc[:],
                                op=OP.add)


TP_MODE = os.environ.get("BERT_TP", "dma")      # 'dma' xbar | 'pe' tensor engine
BCAST_MODE = os.environ.get("BERT_BCAST", "gpsimd")  # 'gpsimd' | 'pe'
# CoreSim lacks Gelu; BERT_SIMACT=1 swaps in Tanh (same dataflow) for sim runs
GELU_AF = AF.Tanh if os.environ.get("BERT_SIMACT") else AF.Gelu


def _dma_transpose_tile(nc, src_bf, dst, tt, pools):
    """src_bf[:, tt*768:(tt+1)*768] (tok-major bf16) -> dst hidden-major cols."""
    if TP_MODE == "dma":
        out_view = dst[:, tt * H:(tt + 1) * H].rearrange("p (c t) -> p c t", c=HC)
        nc.sync.dma_start_transpose(out_view, src_bf[:, tt * H:(tt + 1) * H])
    else:
        psT, ident = pools["psT"], pools["ident"]
        for c in range(HC):
            tp = psT.tile([128, 128], bf16, tag="tp", name="tp")
            nc.tensor.transpose(tp[:],
                                src_bf[:, tt * H + c * 128: tt * H + c * 128 + 128],
                                ident[:])
            nc.vector.tensor_copy(
                dst[:, tt * H + c * 128: tt * H + c * 128 + 128], tp[:])


def build(n_layers=L, flags=None):
    fl = flags or {}
    qk_bias = fl.get("qk_bias", True)
    v_bias = fl.get("v_bias", True)
    o_bias = fl.get("o_bias", True)
    i_bias = fl.get("i_bias", True)
    d_bias = fl.get("d_bias", True)
    ln1_aff = fl.get("ln1_aff", True)
    ln2_aff = fl.get("ln2_aff", True)
    emb_aff = fl.get("emb_aff", True)
    use_mask = fl.get("use_mask", True)
    use_type = fl.get("use_type", True)

    nc = bacc.Bacc("TRN2", target_bir_lowering=False, debug=False, num_devices=8)

    dt_in = lambda n, s, d: nc.dram_tensor(n, s, d, kind="ExternalInput").ap()
    ids_d = dt_in("ids", [S], i32)
    tti_d = dt_in("tti", [S], i32)
    mb_d = dt_in("mb", [S], f32)
    tok_d = dt_in("tok_emb", [V, H], f32)
    pos_d = dt_in("pos_emb", [S, H], f32)
    typ_d = dt_in("type_emb", [2, H], f32)
    eg_d = dt_in("emb_g", [H], f32)
    eb_d = dt_in("emb_b", [H], f32)
    wq_d = dt_in("WqS", [L, HC, 128, HC, 128], bf16)
    wk_d = dt_in("WkS", [L, HC, 128, HC, 128], bf16)
    wv_d = dt_in("WvS", [L, 2, 128, HC, 384], bf16)
    wo_d = dt_in("WoS", [L, 2, 128, HC, 384], bf16)
    wi_d = dt_in("WiS", [L, FC, 128, HC, 128], bf16)
    wd_d = dt_in("WdB", [L, FC // 4, 128, 4, H], bf16)
    bq_d = dt_in("bq", [L, H], f32)
    bk_d = dt_in("bk", [L, H], f32)
    bv_d = dt_in("bv", [L, H], f32)
    bo_d = dt_in("bo", [L, H], f32r)
    bi_d = dt_in("bi", [L, I], f32)
    bd_d = dt_in("bd", [L, H], f32r)
    g1_d = dt_in("ln1_g", [L, H], f32)
    b1_d = dt_in("ln1_b", [L, H], f32)
    g2_d = dt_in("ln2_g", [L, H], f32)
    b2_d = dt_in("ln2_b", [L, H], f32)
    ones_d = dt_in("ones", [128, 128], f32r)
    ident_d = dt_in("ident", [128, 128], bf16)
    out_d = nc.dram_tensor("out", [S, H], f32, kind="ExternalOutput").ap()
    DBG = bool(os.environ.get("BERT_DBG"))
    dbg = {}
    if DBG:
        for nm, w, dt in (("x_bf", TT * H, bf16), ("xT", TT * H, bf16),
                          ("QT", TT * H, bf16), ("KT", TT * H, bf16),
                          ("V65", TT * NH * HD, bf16), ("attnT", TT * H, bf16),
                          ("y", TT * H, f32), ("y_bf", TT * H, bf16),
                          ("yT", TT * H, bf16), ("h1T", FC * S, bf16),
                          ("ffnout", TT * H, f32), ("es00", S, bf16), ("es01", S, bf16), ("es02", S, bf16), ("es03", S, bf16),
                          ("av0", S, f32), ("rd0", S, f32), ("bcs0", S, f32)):
            dbg[nm] = nc.dram_tensor(f"dbg_{nm}", [128, w], dt,
                                     kind="ExternalOutput").ap()

    def dump(nm, t):
        if DBG:
            nc.sync.dma_start(dbg[nm][:], t[:])

    with tile.TileContext(nc) as tc, ExitStack() as ctx:
        tb = ctx.enter_context(tc.tile_pool(name="tb", bufs=5))       # bf16 hidden-major
        resf = ctx.enter_context(tc.tile_pool(name="resf", bufs=2))   # f32 residual
        resbf = ctx.enter_context(tc.tile_pool(name="resbf", bufs=2))  # bf16 post-LN
        v65p = ctx.enter_context(tc.tile_pool(name="v65p", bufs=1))
        h1p = ctx.enter_context(tc.tile_pool(name="h1p", bufs=1))
        wbig = ctx.enter_context(tc.tile_pool(name="wbig", bufs=4))
        wsmall = ctx.enter_context(tc.tile_pool(name="wsmall", bufs=6))
        wdp = ctx.enter_context(tc.tile_pool(name="wdp", bufs=3))
        gb = ctx.enter_context(tc.tile_pool(name="gb", bufs=2))
        exps_p = ctx.enter_context(tc.tile_pool(name="exps_p", bufs=12))
        rd_p = ctx.enter_context(tc.tile_pool(name="rd_p", bufs=2))
        scratch = ctx.enter_context(tc.tile_pool(name="scratch", bufs=1))
        vec = ctx.enter_context(tc.tile_pool(name="vec", bufs=24))
        stats = ctx.enter_context(tc.tile_pool(name="stats", bufs=3))
        brow_p = ctx.enter_context(tc.tile_pool(name="brow_p", bufs=1))
        const = ctx.enter_context(tc.tile_pool(name="const", bufs=1))
        psV_bufs = 1 if TP_MODE == "pe" else 2
        psA_bufs = (8 - psV_bufs - (2 if TP_MODE == "pe" else 0)
                    - (1 if BCAST_MODE == "pe" else 0))
        psA = ctx.enter_context(tc.tile_pool(name="psA", bufs=psA_bufs, space="PSUM"))
        psV = ctx.enter_context(tc.tile_pool(name="psV", bufs=psV_bufs, space="PSUM"))
        if TP_MODE == "pe":
            psT = ctx.enter_context(tc.tile_pool(name="psT", bufs=2, space="PSUM"))
        if BCAST_MODE == "pe":
            psB = ctx.enter_context(tc.tile_pool(name="psB", bufs=1, space="PSUM"))
        pools = dict(gb=gb, vec=vec)
        pools["sq_scratch"] = scratch.tile([128, H], f32, tag="sq", name="sq_scratch")

        # constants
        ones_sb = const.tile([128, 128], f32r, tag="ones", name="ones_sb")
        nc.sync.dma_start(ones_sb[:], ones_d[:])
        if TP_MODE == "pe":
            ident = const.tile([128, 128], bf16, tag="ident", name="ident")
            nc.sync.dma_start(ident[:], ident_d[:])
            pools["psT"] = psT
            pools["ident"] = ident
        if BCAST_MODE == "pe":
            ones_bf = const.tile([1, 64], bf16, tag="ones_bf", name="ones_bf")
            nc.vector.memset(ones_bf[:], 1.0)
        ids_sb = const.tile([128, TT], i32, tag="ids", name="ids_sb")
        nc.sync.dma_start(ids_sb[:], ids_d.rearrange("(t p) -> p t", p=128))
        if use_type:
            tti_sb = const.tile([128, TT], i32, tag="tti", name="tti_sb")
            nc.sync.dma_start(tti_sb[:], tti_d.rearrange("(t p) -> p t", p=128))
        if use_mask:
            mb_sb = const.tile([128, TT], f32, tag="mb", name="mb_sb")
            nc.sync.dma_start(mb_sb[:], mb_d.rearrange("(t p) -> p t", p=128))

        _act_preload(nc, pools, AF.Exp)  # expln table resident from the start

        # ---- embedding -> x (f32) -> LN -> x_bf (bf16) ----
        x_emb = resf.tile([128, TT * H], f32, tag="res", name="x_emb")
        eg_bc, eb_bc = _ln_bcast(nc, pools, eg_d, eb_d, emb_aff)
        s_emb = stats.tile([128, TT], f32, tag="s", name="s_emb")
        ssq_emb = stats.tile([128, TT], f32, tag="ssq", name="ssq_emb")
        x_bf = resbf.tile([128, TT * H], bf16, tag="rbf", name="x_bf_emb")
        for tt in range(TT):
            sl = slice(tt * H, (tt + 1) * H)
            nc.gpsimd.indirect_dma_start(
                out=x_emb[:, sl], out_offset=None, in_=tok_d[:],
                in_offset=bass.IndirectOffsetOnAxis(ap=ids_sb[:, tt:tt + 1], axis=0))
            if use_type:
                tmp_t = gb.tile([128, H], f32, tag="gb", name="emb_tmp")
                nc.gpsimd.indirect_dma_start(
                    out=tmp_t[:], out_offset=None, in_=typ_d[:],
                    in_offset=bass.IndirectOffsetOnAxis(ap=tti_sb[:, tt:tt + 1], axis=0))
                nc.vector.tensor_tensor(out=x_emb[:, sl], in0=x_emb[:, sl],
                                        in1=tmp_t[:], op=OP.add)
            tmp_p = gb.tile([128, H], f32, tag="gb", name="emb_pos")
            nc.sync.dma_start(tmp_p[:], pos_d[tt * 128:(tt + 1) * 128, :])
            nc.vector.tensor_tensor(out=x_emb[:, sl], in0=x_emb[:, sl],
                                    in1=tmp_p[:], op=OP.add)
            nc.vector.reduce_sum(out=s_emb[:, tt:tt + 1], in_=x_emb[:, sl],
                                 axis=AX.X)
            _ln_tile(nc, pools, x_emb, x_bf, tt, s_emb[:, tt:tt + 1],
                     ssq_emb[:, tt:tt + 1], eg_bc, eb_bc)

        # ---- layers ----
        dump("x_bf", x_bf)
        for l in range(n_layers):
            last = (l == n_layers - 1)

            xT = tb.tile([128, HC * S], bf16, tag="t", name=f"xT_{l}")
            for tt in range(TT):
                _dma_transpose_tile(nc, x_bf, xT, tt, pools)
            if l > 0:
                _act_preload(nc, pools, AF.Exp)  # table for this layer's exps
            if l == 0:
                dump("xT", xT)

            # Q^T, K^T hidden-major; token-halves so early chunks start sooner
            QT = tb.tile([128, HC * S], bf16, tag="t", name=f"QT_{l}")
            KT = tb.tile([128, HC * S], bf16, tag="t", name=f"KT_{l}")
            for dst, w_d, b_d in ((QT, wq_d, bq_d), (KT, wk_d, bk_d)):
                for j in range(HC):
                    wblk = wsmall.tile([128, HC, 128], bf16, tag="ws", name="wqk_blk")
                    nc.sync.dma_start(wblk[:], w_d[l, j])
                    pq = psA.tile([128, S], f32, tag="main", name="pq")
                    xTv = xT.rearrange("p (f c t) -> p f c t", f=TT, c=HC)
                    for ic in range(HC):
                        nc.tensor.matmul(
                            pq[:], lhsT=wblk[:, ic, :], rhs=xTv[:, :, ic, :],
                            start=(ic == 0), stop=(ic == HC - 1))
                    jsl = slice(j * S, (j + 1) * S)
                    if qk_bias:
                        b_sl = vec.tile([128, 1], f32, tag="v", name="bqk_sl")
                        nc.sync.dma_start(b_sl[:], b_d[l, j * 128:(j + 1) * 128][:, None])
                        nc.scalar.activation(dst[:, jsl], pq[:], AF.Identity,
                                             bias=b_sl[:])
                    else:
                        nc.vector.tensor_copy(dst[:, jsl], pq[:])

            if l == 0:
                dump("QT", QT)
                dump("KT", KT)
            # V token-major with a ones column per head (denominator fold)
            V65t = v65p.tile([128, TT * NH * HD], bf16, tag="v65", name=f"V65_{l}")
            ones_cols = V65t.rearrange("p (f h d) -> p f h d", f=TT, h=NH)[:, :, :, D]
            nc.vector.memset(ones_cols, 1.0)
            for n in range(2):
                wvblk = wbig.tile([128, HC, 384], bf16, tag="wb", name="wv_blk")
                nc.sync.dma_start(wvblk[:], wv_d[l, n])
                for tt in range(TT):
                    pv = psA.tile([128, 384], f32, tag="main", name="pv")
                    for ic in range(HC):
                        nc.tensor.matmul(
                            pv[:], lhsT=xT[:, tt * H + ic * 128: tt * H + ic * 128 + 128],
                            rhs=wvblk[:, ic, :],
                            start=(ic == 0), stop=(ic == HC - 1))
                    # strided copy: 6 heads' 64-wide blocks into 65-wide slots
                    dst = V65t.rearrange("p (f h d) -> p f h d", f=TT, h=NH)[
                        :, tt, n * 6:(n + 1) * 6, 0:D]
                    src = pv.rearrange("p (h d) -> p h d", h=6)
                    nc.vector.tensor_copy(dst, src)

            if l == 0:
                dump("V65", V65t)
            # attention, head pairs
            attnT = tb.tile([128, HC * S], bf16, tag="t", name=f"attnT_{l}")
            v65v = V65t.rearrange("p (f h d) -> p f h d", f=TT, h=NH)
            for c in range(HC):
                es = [[None] * TT for _ in range(2)]
                for hh in range(2):
                    r0 = 64 * hh
                    for kc in range(TT):
                        sp = psA.tile([128, S], f32, tag="main", name="sp")
                        nc.tensor.matmul(
                            sp[:],
                            lhsT=KT[r0:r0 + 64, c * S + kc * 128: c * S + kc * 128 + 128],
                            rhs=QT[r0:r0 + 64, c * S:(c + 1) * S],
                            start=True, stop=True)
                        e = exps_p.tile([128, S], bf16, tag="e", name=f"e{hh}_{kc}")
                        mbias = mb_sb[:, kc:kc + 1] if use_mask else 0.0
                        nc.scalar.activation(e[:], sp[:], AF.Exp,
                                             bias=mbias, scale=0.125)
                        es[hh][kc] = e
                for hh in range(2):
                    h = 2 * c + hh
                    av = psV.tile([HD, S], f32, tag="av", name="av")
                    for kc in range(TT):
                        nc.tensor.matmul(
                            av[:], lhsT=v65v[:, kc, h, :], rhs=es[hh][kc][:],
                            start=(kc == 0), stop=(kc == TT - 1))
                    # regular-op reciprocal: a custom-DVE read of a PSUM
                    # accumulation group races ahead of the group's tail
                    rd = rd_p.tile([1, S], f32, tag="rd", name="rd")
                    nc.vector.reciprocal(rd[:], av[D:HD, :])
                    if DBG and l == 0 and c == 0 and hh == 0:
                        for _kc in range(TT):
                            nc.sync.dma_start(dbg[f"es0{_kc}"][0:128, :],
                                              es[0][_kc][:])
                        avd = gb.tile([128, S], f32, tag="gb", name="avd")
                        nc.vector.tensor_copy(avd[0:HD, :], av[:])
                        nc.sync.dma_start(dbg["av0"][0:HD, :], avd[0:HD, :])
                        nc.sync.dma_start(dbg["rd0"][0:1, :], rd[:])
                    bcs = rd_p.tile([64, S], f32, tag="bcs", name="bcs")
                    if BCAST_MODE == "gpsimd":
                        nc.gpsimd.partition_broadcast(bcs[:], rd[0:1, :])
                    else:
                        rdb = rd_p.tile([1, S], bf16, tag="rdb", name="rdb")
                        nc.vector.tensor_copy(rdb[:], rd[:])
                        bcp = psB.tile([64, S], f32, tag="bc", name="bcp")
                        nc.tensor.matmul(bcp[:], lhsT=ones_bf[0:1, 0:64],
                                         rhs=rdb[:], start=True, stop=True)
                        nc.vector.tensor_copy(bcs[:], bcp[:])
                    if DBG and l == 0 and c == 0 and hh == 0:
                        nc.sync.dma_start(dbg["bcs0"][0:64, :], bcs[:])
                    dst = attnT[64 * hh:64 * hh + 64, c * S:(c + 1) * S]
                    nc.vector.tensor_tensor(out=dst, in0=av[0:D, :], in1=bcs[:],
                                            op=OP.mult)
                    if v_bias:
                        bv_sl = vec.tile([64, 1], f32, tag="bv", name="bv_sl")
                        nc.sync.dma_start(bv_sl[:], bv_d[l, h * D:(h + 1) * D][:, None])
                        nc.vector.tensor_scalar(
                            out=dst, in0=dst,
                            scalar1=bv_sl[:], scalar2=None, op0=OP.add)

            if l == 0:
                dump("attnT", attnT)
            _act_preload(nc, pools, AF.Sqrt)
            # Wo projection (+bo) + residual -> y (f32), LN1 -> y_bf (bf16)
            y = resf.tile([128, TT * H], f32, tag="res", name=f"y_{l}")
            y_bf = resbf.tile([128, TT * H], bf16, tag="rbf", name=f"ybf_{l}")
            g1_bc, b1_bc = _ln_bcast(nc, pools, g1_d[l], b1_d[l], ln1_aff)
            s1 = stats.tile([128, TT], f32, tag="s", name=f"s1_{l}")
            ssq1 = stats.tile([128, TT], f32, tag="ssq", name=f"ssq1_{l}")
            if o_bias:
                bo_row = brow_p.tile([1, H], f32r, tag="br", name="bo_row")
                nc.sync.dma_start(bo_row[:], bo_d[l][None, :])
            woblks = []
            for n in range(2):
                wob = wbig.tile([128, HC, 384], bf16, tag="wb", name=f"wo_blk{n}")
                nc.sync.dma_start(wob[:], wo_d[l, n])
                woblks.append(wob)
            for tt in range(TT):
                for n in range(2):
                    po = psA.tile([128, 384], f32, tag="main", name="po")
                    if o_bias:
                        nc.tensor.matmul(po[:], lhsT=ones_sb[0:1, 0:128],
                                         rhs=bo_row[0:1, n * 384:(n + 1) * 384],
                                         start=True, stop=False)
                    for jc in range(HC):
                        nc.tensor.matmul(
                            po[:],
                            lhsT=attnT[:, jc * S + tt * 128: jc * S + tt * 128 + 128],
                            rhs=woblks[n][:, jc, :],
                            start=(not o_bias and jc == 0), stop=(jc == HC - 1))
                    sl = slice(tt * H + n * 384, tt * H + n * 384 + 384)
                    nc.vector.tensor_tensor(out=y[:, sl], in0=po[:, :],
                                            in1=x_bf[:, sl], op=OP.add)
                nc.vector.reduce_sum(out=s1[:, tt:tt + 1],
                                     in_=y[:, tt * H:(tt + 1) * H], axis=AX.X)
                _ln_tile(nc, pools, y, y_bf, tt, s1[:, tt:tt + 1],
                         ssq1[:, tt:tt + 1], g1_bc, b1_bc)

            if l == 0:
                dump("y", y)
                dump("y_bf", y_bf)
            # yT (DMA xbar), then preload the Gelu table while matmuls run
            yT = tb.tile([128, HC * S], bf16, tag="t", name=f"yT_{l}")
            for tt in range(TT):
                _dma_transpose_tile(nc, y_bf, yT, tt, pools)
            _act_preload(nc, pools, GELU_AF)
            if l == 0:
                dump("yT", yT)

            # FFN up: h1T = gelu(yT @ Wi + bi), hidden-major, bf16
            h1T = h1p.tile([128, FC * S], bf16, tag="h1", name=f"h1T_{l}")
            for fc in range(FC):
                wiblk = wsmall.tile([128, HC, 128], bf16, tag="ws", name="wi_blk")
                nc.sync.dma_start(wiblk[:], wi_d[l, fc])
                ph = psA.tile([128, S], f32, tag="main", name="ph")
                yTv = yT.rearrange("p (f c t) -> p f c t", f=TT, c=HC)
                for ic in range(HC):
                    nc.tensor.matmul(
                        ph[:], lhsT=wiblk[:, ic, :], rhs=yTv[:, :, ic, :],
                        start=(ic == 0), stop=(ic == HC - 1))
                if i_bias:
                    bi_sl = vec.tile([128, 1], f32, tag="v", name="bi_sl")
                    nc.sync.dma_start(bi_sl[:], bi_d[l, fc * 128:(fc + 1) * 128][:, None])
                    nc.scalar.activation(h1T[:, fc * S:(fc + 1) * S], ph[:], GELU_AF,
                                         bias=bi_sl[:])
                else:
                    nc.scalar.activation(h1T[:, fc * S:(fc + 1) * S], ph[:], GELU_AF)

            if l == 0:
                dump("h1T", h1T)
            # FFN down (bf16) + bd + residual -> ffnout; waves of 4 (tt,n) pairs
            ffnout = resf.tile([128, TT * H], f32, tag="res", name=f"ffnout_{l}")
            xbf_next = (None if last else
                        resbf.tile([128, TT * H], bf16, tag="rbf", name=f"xbf_{l + 1}"))
            g2_bc, b2_bc = _ln_bcast(nc, pools, g2_d[l], b2_d[l], ln2_aff)
            s2 = stats.tile([128, TT], f32, tag="s", name=f"s2_{l}")
            ssq2 = stats.tile([128, TT], f32, tag="ssq", name=f"ssq2_{l}")
            if d_bias:
                bd_row = brow_p.tile([1, H], f32r, tag="br", name="bd_row")
                nc.sync.dma_start(bd_row[:], bd_d[l][None, :])
            for wave in range(2):
                tts = (0, 1) if wave == 0 else (2, 3)
                wave_pairs = [(tt, n) for tt in tts for n in range(2)]
                accs = {}
                for (tt, n) in wave_pairs:
                    acc = psA.tile([128, 384], f32, tag="main", name=f"acc{tt}_{n}")
                    if d_bias:
                        nc.tensor.matmul(acc[:], lhsT=ones_sb[0:1, 0:128],
                                         rhs=bd_row[0:1, n * 384:(n + 1) * 384],
                                         start=True, stop=False)
                    accs[(tt, n)] = acc
                for fp in range(FC // 4):
                    wdblk = wdp.tile([128, 4, H], bf16, tag="wd", name="wd_blk")
                    nc.sync.dma_start(wdblk[:], wd_d[l, fp])
                    for two in range(4):
                        fc = 4 * fp + two
                        for (tt, n) in wave_pairs:
                            nc.tensor.matmul(
                                accs[(tt, n)][:],
                                lhsT=h1T[:, fc * S + tt * 128: fc * S + tt * 128 + 128],
                                rhs=wdblk[:, two, n * 384:(n + 1) * 384],
                                start=(not d_bias and fc == 0), stop=(fc == FC - 1))
                if wave == 0:
                    # sqrt table back in residence before LN2's rstd
                    _act_preload(nc, pools, AF.Sqrt)
                for tt in tts:
                    for n in range(2):
                        sl = slice(tt * H + n * 384, tt * H + n * 384 + 384)
                        nc.vector.tensor_tensor(out=ffnout[:, sl],
                                                in0=accs[(tt, n)][:, :],
                                                in1=y_bf[:, sl], op=OP.add)
                    nc.vector.reduce_sum(out=s2[:, tt:tt + 1],
                                         in_=ffnout[:, tt * H:(tt + 1) * H],
                                         axis=AX.X)
                    if last:
                        x_out = pools.get("x_out")
                        if x_out is None:
                            x_out = scratch.tile([128, TT * H], f32, tag="xo",
                                                 name="x_out")
                            pools["x_out"] = x_out
                        _ln_tile(nc, pools, ffnout, x_out, tt, s2[:, tt:tt + 1],
                                 ssq2[:, tt:tt + 1], g2_bc, b2_bc)
                        nc.sync.dma_start(
                            out_d[tt * 128:(tt + 1) * 128, :],
                            x_out[:, tt * H:(tt + 1) * H])
                    else:
                        _ln_tile(nc, pools, ffnout, xbf_next, tt, s2[:, tt:tt + 1],
                                 ssq2[:, tt:tt + 1], g2_bc, b2_bc)
            if l == 0:
                dump("ffnout", ffnout)
            x_bf = xbf_next

    nc.compile()
    return nc


def _prep_inputs(inputs, b):
    f = np.float32
    bh = ml_dtypes.bfloat16
    Wq, Wk, Wv, Wo, Wi = (np.asarray(inputs[k], f) for k in ("Wq", "Wk", "Wv", "Wo", "Wi"))
    WqS = np.ascontiguousarray(
        Wq.reshape(L, HC, 128, HC, 128).transpose(0, 3, 2, 1, 4)).astype(bh)
    WkS = np.ascontiguousarray(
        Wk.reshape(L, HC, 128, HC, 128).transpose(0, 3, 2, 1, 4)).astype(bh)
    WvS = np.ascontiguousarray(
        Wv.reshape(L, HC, 128, 2, 384).transpose(0, 3, 2, 1, 4)).astype(bh)
    WoS = np.ascontiguousarray(
        Wo.reshape(L, HC, 128, 2, 384).transpose(0, 3, 2, 1, 4)).astype(bh)
    WiS = np.ascontiguousarray(
        Wi.reshape(L, HC, 128, FC, 128).transpose(0, 3, 2, 1, 4)).astype(bh)
    Wd = np.asarray(inputs["Wd"], f)
    WdB = np.ascontiguousarray(
        Wd.reshape(L, FC // 4, 4, 128, H).transpose(0, 1, 3, 2, 4)).astype(bh)
    mask = np.asarray(inputs["input_mask"], f)
    tti = np.asarray(inputs["token_type_ids"], np.int32)
    flags = dict(
        qk_bias=bool(np.any(np.asarray(inputs["bq"])) or np.any(np.asarray(inputs["bk"]))),
        v_bias=bool(np.any(np.asarray(inputs["bv"]))),
        o_bias=bool(np.any(np.asarray(inputs["bo"]))),
        i_bias=bool(np.any(np.asarray(inputs["bi"]))),
        d_bias=bool(np.any(np.asarray(inputs["bd"]))),
        ln1_aff=bool(np.any(np.asarray(inputs["ln1_g"]) != 1.0) or
                     np.any(np.asarray(inputs["ln1_b"]))),
        ln2_aff=bool(np.any(np.asarray(inputs["ln2_g"]) != 1.0) or
                     np.any(np.asarray(inputs["ln2_b"]))),
        emb_aff=bool(np.any(np.asarray(inputs["emb_ln_g"]) != 1.0) or
                     np.any(np.asarray(inputs["emb_ln_b"]))),
        use_mask=bool(np.any(mask != 1.0)),
        use_type=bool(np.any(tti != 0)),
    )
    pos_eff = np.asarray(inputs["pos_emb"], f)[:S]
    if not flags["use_type"]:
        pos_eff = pos_eff + np.asarray(inputs["type_emb"], f)[int(tti.flat[0])][None, :]
    shared = dict(
        tok_emb=np.asarray(inputs["tok_emb"], f),
        pos_emb=pos_eff,
        type_emb=np.asarray(inputs["type_emb"], f),
        emb_g=np.asarray(inputs["emb_ln_g"], f),
        emb_b=np.asarray(inputs["emb_ln_b"], f),
        WqS=WqS, WkS=WkS, WvS=WvS, WoS=WoS, WiS=WiS, WdB=WdB,
        bq=np.asarray(inputs["bq"], f), bk=np.asarray(inputs["bk"], f),
        bv=np.asarray(inputs["bv"], f), bo=np.asarray(inputs["bo"], f),
        bi=np.asarray(inputs["bi"], f), bd=np.asarray(inputs["bd"], f),
        ln1_g=np.asarray(inputs["ln1_g"], f), ln1_b=np.asarray(inputs["ln1_b"], f),
        ln2_g=np.asarray(inputs["ln2_g"], f), ln2_b=np.asarray(inputs["ln2_b"], f),
        ones=np.ones((128, 128), f),
        ident=np.eye(128, dtype=ml_dtypes.bfloat16),
    )
    in_maps = []
    ids = np.asarray(inputs["input_ids"], np.int32)
    for c in range(b):
        m = dict(shared)
        m["ids"] = np.ascontiguousarray(ids[c])
        m["tti"] = np.ascontiguousarray(tti[c])
        m["mb"] = np.ascontiguousarray((1.0 - mask[c]) * -10000.0)
        in_maps.append(m)
    return in_maps, flags


def kernel(**inputs):
    global LAST_EXEC_TIME_NS
    n_layers = int(os.environ.get("BERT_LAYERS", L))
    trace = bool(os.environ.get("BERT_TRACE"))
    in_maps, flags = _prep_inputs(inputs, B)
    nc = build(n_layers, flags)
    res = bass_utils.run_bass_kernel_spmd(
        nc, in_maps, core_ids=list(range(B)), trace=trace)
    LAST_EXEC_TIME_NS = res.exec_time_ns
    out = np.stack([res.results[c]["out"] for c in range(B)])
    return out.astype(np.float32)


# revision 27
# speedup vs baseline: 1.1398x; 1.0653x over previous
"""BERT-base forward on 8 Trainium2 NeuronCores, data-parallel over batch.

Each core runs the full 12-layer model on one batch element (512 tokens).
All matmul operands are bf16 (weights pre-cast on host, activations cast at
PSUM eviction); the residual/LN stream stays f32.  PE weight loads serialize
with matmuls on TRN2, so bf16 halves both HBM traffic and LDWEIGHTS time.

Layouts per core (SBUF tiles are [128 partitions, free]):
  token-major  y/ffnout (f32) and x_bf/y_bf (bf16): [128 tok, 4*768]
  hidden-major xT/QT/KT/attnT/yT (bf16): [128 hid, 6*512]
  V65 token-major bf16: [128 tok, 4*(12*65)] — 64 value dims + a ones
  column per head, so the attention-value matmul also produces the softmax
  denominator as output partition 64 (no separate ones-matmul pass).
  h1T hidden-major [128 f, 24*512] bf16.

Attention (per head pair c: heads 2c at partitions 0:64, 2c+1 at 64:128):
  S^T[k,q] = matmul(lhsT=KT[d,k-tile], rhs=QT[d,q]) row-packed pairs
  expS = Exp(S^T/8 + maskbias_k) -> bf16  (|scores/8| < 3, no max needed)
  av[0:65] = matmul(lhsT=V65, rhs=expS) summed over k chunks; row 64 = denom
  rd = approx 1/denom (DVE) -> broadcast to 64 partitions via a tiny matmul
  attnT = av[0:64] * rd_bc  (+bv) -> bf16

Token-major<->hidden-major transposes run on the DMA xbar
(dma_start_transpose, 2-byte dtype), not the PE.

LayerNorm: row sums accumulate for free during the residual-add eviction
(tensor_tensor_reduce); sum-of-squares via ACT Square+accum; rstd =
exp(-0.5*ln(var+eps)) so Exp/Ln/Square share one activation table and the
only table switches per layer are Gelu<->Exp, both preloaded off the
critical path.

Work that is provably a no-op for the given inputs (zero biases, unit
gammas, zero betas, all-ones mask) is skipped at build time; the general
path stays available and is selected per-input on the host.
"""
import os
import numpy as np
import ml_dtypes
from contextlib import ExitStack

import concourse.bass as bass
import concourse.tile as tile
from concourse import bacc, mybir
from concourse import bass_utils

f32 = mybir.dt.float32
f32r = mybir.dt.float32r
bf16 = mybir.dt.bfloat16
i32 = mybir.dt.int32
AF = mybir.ActivationFunctionType
OP = mybir.AluOpType
AX = mybir.AxisListType

V, H, L, NH, I, P, B, S = 30000, 768, 12, 12, 3072, 512, 8, 512
D = H // NH          # 64
HC = H // 128        # 6 hidden chunks
FC = I // 128        # 24 ffn chunks
TT = S // 128        # 4 token tiles
HD = D + 1           # 65: value dims + denominator ones column
LN_EPS = 1e-3

LAST_EXEC_TIME_NS = None


def _act_preload(nc, pools, func):
    """Touch `func` on a dummy so its table load lands off the critical path."""
    vec = pools["vec"]
    j = vec.tile([128, 1], f32, tag="pre", name="act_pre")
    nc.vector.memset(j[:], 1.0)
    nc.scalar.activation(j[:], j[:], func)


def _ln_bcast(nc, pools, g_row, b_row, affine):
    if not affine:
        return None, None
    gb = pools["gb"]
    g_bc = gb.tile([128, H], f32, tag="gb", name="g_bc")
    nc.sync.dma_start(g_bc[:], g_row[None, :].partition_broadcast(128))
    b_bc = gb.tile([128, H], f32, tag="gb", name="b_bc")
    nc.sync.dma_start(b_bc[:], b_row[None, :].partition_broadcast(128))
    return g_bc, b_bc


def _ln_stats_sq(nc, pools, z, tt, n, ssq_col):
    """Sum-of-squares for 384-col chunk n of tile tt (emit right at evict)."""
    sl = slice(tt * H + n * 384, tt * H + n * 384 + 384)
    nc.scalar.activation(pools["sq_scratch"][:, 0:384], z[:, sl], AF.Square,
                         accum_out=ssq_col)


def _ln_tile(nc, pools, z, zout, tt, sA, sB, qA, qB, g_bc, b_bc):
    """LN tile tt of z (f32) -> zout tile (bf16/f32).

    Row sums and sum-of-squares arrive as per-384-chunk partials (computed
    at eviction time), so only a short scalar chain remains on the
    boundary-critical path. var = E[x^2] - mu^2.
    """
    vec = pools["vec"]
    sl = slice(tt * H, (tt + 1) * H)
    s = vec.tile([128, 1], f32, tag="v", name=f"ln_s_{tt}")
    nc.vector.tensor_tensor(out=s[:], in0=sA, in1=sB, op=OP.add)
    b2 = vec.tile([128, 1], f32, tag="v", name=f"ln_b2_{tt}")
    nc.vector.tensor_scalar(out=b2[:], in0=s[:], scalar1=s[:],
                            scalar2=float(-1.0 / (H * H)), op0=OP.mult,
                            op1=OP.mult)
    nc.vector.tensor_scalar(out=b2[:], in0=b2[:], scalar1=float(LN_EPS),
                            scalar2=None, op0=OP.add)
    b2p = vec.tile([128, 1], f32, tag="v", name=f"ln_b2p_{tt}")
    nc.vector.scalar_tensor_tensor(out=b2p[:], in0=qA, scalar=float(1.0 / H),
                                   in1=b2[:], op0=OP.mult, op1=OP.add)
    sd = vec.tile([128, 1], f32, tag="v", name=f"ln_sd_{tt}")
    nc.scalar.activation(sd[:], qB, AF.Sqrt, bias=b2p[:], scale=1.0 / H)
    rstd = vec.tile([128, 1], f32, tag="v", name=f"ln_rstd_{tt}")
    nc.vector.reciprocal_approx_fast(out=rstd[:], in_=sd[:])
    mr = vec.tile([128, 1], f32, tag="v", name=f"ln_mr_{tt}")
    nc.vector.tensor_scalar(out=mr[:], in0=s[:], scalar1=rstd[:],
                            scalar2=float(-1.0 / H), op0=OP.mult, op1=OP.mult)
    nc.vector.tensor_scalar(out=zout[:, sl], in0=z[:, sl], scalar1=rstd[:],
                            scalar2=mr[:], op0=OP.mult, op1=OP.add)
    if g_bc is not None:
        nc.vector.tensor_tensor(out=zout[:, sl], in0=zout[:, sl], in1=g_bc[:],
                                op=OP.mult)
    if b_bc is not None:
        nc.vector.tensor_tensor(out=zout[:, sl], in0=zout[:, sl], in1=b_bc[:],
                                op=OP.add)


TP_MODE = os.environ.get("BERT_TP", "dma")      # 'dma' xbar | 'pe' tensor engine
BCAST_MODE = os.environ.get("BERT_BCAST", "gpsimd")  # 'gpsimd' | 'pe'
# CoreSim lacks Gelu; BERT_SIMACT=1 swaps in Tanh (same dataflow) for sim runs
GELU_AF = AF.Tanh if os.environ.get("BERT_SIMACT") else AF.Gelu


def _dma_transpose_tile(nc, src_bf, dst, tt, pools):
    """src_bf[:, tt*768:(tt+1)*768] (tok-major bf16) -> dst hidden-major cols."""
    if TP_MODE == "dma":
        out_view = dst[:, tt * H:(tt + 1) * H].rearrange("p (c t) -> p c t", c=HC)
        nc.sync.dma_start_transpose(out_view, src_bf[:, tt * H:(tt + 1) * H])
    else:
        psT, ident = pools["psT"], pools["ident"]
        for c in range(HC):
            tp = psT.tile([128, 128], bf16, tag="tp", name="tp")
            nc.tensor.transpose(tp[:],
                                src_bf[:, tt * H + c * 128: tt * H + c * 128 + 128],
                                ident[:])
            nc.vector.tensor_copy(
                dst[:, tt * H + c * 128: tt * H + c * 128 + 128], tp[:])


def build(n_layers=L, flags=None):
    fl = flags or {}
    qk_bias = fl.get("qk_bias", True)
    v_bias = fl.get("v_bias", True)
    o_bias = fl.get("o_bias", True)
    i_bias = fl.get("i_bias", True)
    d_bias = fl.get("d_bias", True)
    ln1_aff = fl.get("ln1_aff", True)
    ln2_aff = fl.get("ln2_aff", True)
    emb_aff = fl.get("emb_aff", True)
    use_mask = fl.get("use_mask", True)
    use_type = fl.get("use_type", True)

    nc = bacc.Bacc("TRN2", target_bir_lowering=False, debug=False, num_devices=8)

    dt_in = lambda n, s, d: nc.dram_tensor(n, s, d, kind="ExternalInput").ap()
    ids_d = dt_in("ids", [S], i32)
    tti_d = dt_in("tti", [S], i32)
    mb_d = dt_in("mb", [S], f32)
    tok_d = dt_in("tok_emb", [V, H], f32)
    pos_d = dt_in("pos_emb", [S, H], f32)
    typ_d = dt_in("type_emb", [2, H], f32)
    eg_d = dt_in("emb_g", [H], f32)
    eb_d = dt_in("emb_b", [H], f32)
    wq_d = dt_in("WqS", [L, HC, 128, HC, 128], bf16)
    wk_d = dt_in("WkS", [L, HC, 128, HC, 128], bf16)
    wv_d = dt_in("WvS", [L, 2, 128, HC, 384], bf16)
    wo_d = dt_in("WoS", [L, 2, 128, HC, 384], bf16)
    wi_d = dt_in("WiS", [L, FC, 128, HC, 128], bf16)
    wd_d = dt_in("WdB", [L, FC // 4, 128, 4, H], bf16)
    bq_d = dt_in("bq", [L, H], f32)
    bk_d = dt_in("bk", [L, H], f32)
    bv_d = dt_in("bv", [L, H], f32)
    bo_d = dt_in("bo", [L, H], f32r)
    bi_d = dt_in("bi", [L, I], f32)
    bd_d = dt_in("bd", [L, H], f32r)
    g1_d = dt_in("ln1_g", [L, H], f32)
    b1_d = dt_in("ln1_b", [L, H], f32)
    g2_d = dt_in("ln2_g", [L, H], f32)
    b2_d = dt_in("ln2_b", [L, H], f32)
    ones_d = dt_in("ones", [128, 128], f32r)
    ident_d = dt_in("ident", [128, 128], bf16)
    out_d = nc.dram_tensor("out", [S, H], f32, kind="ExternalOutput").ap()
    DBG = bool(os.environ.get("BERT_DBG"))
    dbg = {}
    if DBG:
        for nm, w, dt in (("x_bf", TT * H, bf16), ("xT", TT * H, bf16),
                          ("QT", TT * H, bf16), ("KT", TT * H, bf16),
                          ("V65", TT * NH * HD, bf16), ("attnT", TT * H, bf16),
                          ("y", TT * H, f32), ("y_bf", TT * H, bf16),
                          ("yT", TT * H, bf16), ("h1T", FC * S, bf16),
                          ("ffnout", TT * H, f32), ("es00", S, bf16), ("es01", S, bf16), ("es02", S, bf16), ("es03", S, bf16),
                          ("av0", S, f32), ("rd0", S, f32), ("bcs0", S, f32)):
            dbg[nm] = nc.dram_tensor(f"dbg_{nm}", [128, w], dt,
                                     kind="ExternalOutput").ap()

    def dump(nm, t):
        if DBG:
            nc.sync.dma_start(dbg[nm][:], t[:])

    with tile.TileContext(nc) as tc, ExitStack() as ctx:
        tb = ctx.enter_context(tc.tile_pool(name="tb", bufs=5))       # bf16 hidden-major
        resf = ctx.enter_context(tc.tile_pool(name="resf", bufs=2))   # f32 residual
        resbf = ctx.enter_context(tc.tile_pool(name="resbf", bufs=2))  # bf16 post-LN
        v65p = ctx.enter_context(tc.tile_pool(name="v65p", bufs=1))
        h1p = ctx.enter_context(tc.tile_pool(name="h1p", bufs=1))
        wbig = ctx.enter_context(tc.tile_pool(name="wbig", bufs=4))
        wsmall = ctx.enter_context(tc.tile_pool(name="wsmall", bufs=6))
        wdp = ctx.enter_context(tc.tile_pool(name="wdp", bufs=3))
        gb = ctx.enter_context(tc.tile_pool(name="gb", bufs=2))
        exps_p = ctx.enter_context(tc.tile_pool(name="exps_p", bufs=12))
        rd_p = ctx.enter_context(tc.tile_pool(name="rd_p", bufs=2))
        scratch = ctx.enter_context(tc.tile_pool(name="scratch", bufs=1))
        vec = ctx.enter_context(tc.tile_pool(name="vec", bufs=24))
        stats = ctx.enter_context(tc.tile_pool(name="stats", bufs=3))
        brow_p = ctx.enter_context(tc.tile_pool(name="brow_p", bufs=1))
        const = ctx.enter_context(tc.tile_pool(name="const", bufs=1))
        psV_bufs = 1 if TP_MODE == "pe" else 2
        psA_bufs = (8 - psV_bufs - (2 if TP_MODE == "pe" else 0)
                    - (1 if BCAST_MODE == "pe" else 0))
        psA = ctx.enter_context(tc.tile_pool(name="psA", bufs=psA_bufs, space="PSUM"))
        psV = ctx.enter_context(tc.tile_pool(name="psV", bufs=psV_bufs, space="PSUM"))
        if TP_MODE == "pe":
            psT = ctx.enter_context(tc.tile_pool(name="psT", bufs=2, space="PSUM"))
        if BCAST_MODE == "pe":
            psB = ctx.enter_context(tc.tile_pool(name="psB", bufs=1, space="PSUM"))
        pools = dict(gb=gb, vec=vec)
        pools["sq_scratch"] = scratch.tile([128, H], f32, tag="sq", name="sq_scratch")

        # constants
        ones_sb = const.tile([128, 128], f32r, tag="ones", name="ones_sb")
        nc.sync.dma_start(ones_sb[:], ones_d[:])
        if TP_MODE == "pe":
            ident = const.tile([128, 128], bf16, tag="ident", name="ident")
            nc.sync.dma_start(ident[:], ident_d[:])
            pools["psT"] = psT
            pools["ident"] = ident
        if BCAST_MODE == "pe":
            ones_bf = const.tile([1, 64], bf16, tag="ones_bf", name="ones_bf")
            nc.vector.memset(ones_bf[:], 1.0)
        ids_sb = const.tile([128, TT], i32, tag="ids", name="ids_sb")
        nc.sync.dma_start(ids_sb[:], ids_d.rearrange("(t p) -> p t", p=128))
        if use_type:
            tti_sb = const.tile([128, TT], i32, tag="tti", name="tti_sb")
            nc.sync.dma_start(tti_sb[:], tti_d.rearrange("(t p) -> p t", p=128))
        if use_mask:
            mb_sb = const.tile([128, TT], f32, tag="mb", name="mb_sb")
            nc.sync.dma_start(mb_sb[:], mb_d.rearrange("(t p) -> p t", p=128))

        _act_preload(nc, pools, AF.Exp)  # expln table resident from the start

        # ---- embedding -> x (f32) -> LN -> x_bf (bf16) ----
        x_emb = resf.tile([128, TT * H], f32, tag="res", name="x_emb")
        eg_bc, eb_bc = _ln_bcast(nc, pools, eg_d, eb_d, emb_aff)
        se = [stats.tile([128, TT], f32, tag=f"s{i}", name=f"s_emb{i}")
              for i in range(2)]
        qe = [stats.tile([128, TT], f32, tag=f"q{i}", name=f"q_emb{i}")
              for i in range(2)]
        x_bf = resbf.tile([128, TT * H], bf16, tag="rbf", name="x_bf_emb")
        for tt in range(TT):
            sl = slice(tt * H, (tt + 1) * H)
            nc.gpsimd.indirect_dma_start(
                out=x_emb[:, sl], out_offset=None, in_=tok_d[:],
                in_offset=bass.IndirectOffsetOnAxis(ap=ids_sb[:, tt:tt + 1], axis=0))
            if use_type:
                tmp_t = gb.tile([128, H], f32, tag="gb", name="emb_tmp")
                nc.gpsimd.indirect_dma_start(
                    out=tmp_t[:], out_offset=None, in_=typ_d[:],
                    in_offset=bass.IndirectOffsetOnAxis(ap=tti_sb[:, tt:tt + 1], axis=0))
                nc.vector.tensor_tensor(out=x_emb[:, sl], in0=x_emb[:, sl],
                                        in1=tmp_t[:], op=OP.add)
            tmp_p = gb.tile([128, H], f32, tag="gb", name="emb_pos")
            nc.sync.dma_start(tmp_p[:], pos_d[tt * 128:(tt + 1) * 128, :])
            for n in range(2):
                nsl = slice(tt * H + n * 384, tt * H + n * 384 + 384)
                nc.vector.scalar_tensor_tensor(
                    out=x_emb[:, nsl], in0=x_emb[:, nsl], scalar=0.0,
                    in1=tmp_p[:, n * 384:(n + 1) * 384], op0=OP.add,
                    op1=OP.add, accum_out=se[n][:, tt:tt + 1])
                _ln_stats_sq(nc, pools, x_emb, tt, n, qe[n][:, tt:tt + 1])
            _ln_tile(nc, pools, x_emb, x_bf, tt, se[0][:, tt:tt + 1],
                     se[1][:, tt:tt + 1], qe[0][:, tt:tt + 1],
                     qe[1][:, tt:tt + 1], eg_bc, eb_bc)

        # ---- layers ----
        dump("x_bf", x_bf)
        for l in range(n_layers):
            last = (l == n_layers - 1)

            xT = tb.tile([128, HC * S], bf16, tag="t", name=f"xT_{l}")
            for tt in range(TT):
                _dma_transpose_tile(nc, x_bf, xT, tt, pools)
            if l > 0:
                _act_preload(nc, pools, AF.Exp)  # table for this layer's exps
            if l == 0:
                dump("xT", xT)

            # Q^T, K^T hidden-major; token-halves so early chunks start sooner
            QT = tb.tile([128, HC * S], bf16, tag="t", name=f"QT_{l}")
            KT = tb.tile([128, HC * S], bf16, tag="t", name=f"KT_{l}")
            for dst, w_d, b_d in ((QT, wq_d, bq_d), (KT, wk_d, bk_d)):
                for j in range(HC):
                    wblk = wsmall.tile([128, HC, 128], bf16, tag="ws", name="wqk_blk")
                    nc.sync.dma_start(wblk[:], w_d[l, j])
                    pq = psA.tile([128, S], f32, tag="main", name="pq")
                    xTv = xT.rearrange("p (f c t) -> p f c t", f=TT, c=HC)
                    for ic in range(HC):
                        nc.tensor.matmul(
                            pq[:], lhsT=wblk[:, ic, :], rhs=xTv[:, :, ic, :],
                            start=(ic == 0), stop=(ic == HC - 1))
                    jsl = slice(j * S, (j + 1) * S)
                    if qk_bias:
                        b_sl = vec.tile([128, 1], f32, tag="v", name="bqk_sl")
                        nc.sync.dma_start(b_sl[:], b_d[l, j * 128:(j + 1) * 128][:, None])
                        nc.scalar.activation(dst[:, jsl], pq[:], AF.Identity,
                                             bias=b_sl[:])
                    else:
                        nc.vector.tensor_copy(dst[:, jsl], pq[:])

            if l == 0:
                dump("QT", QT)
                dump("KT", KT)
            # V token-major with a ones column per head (denominator fold)
            V65t = v65p.tile([128, TT * NH * HD], bf16, tag="v65", name=f"V65_{l}")
            ones_cols = V65t.rearrange("p (f h d) -> p f h d", f=TT, h=NH)[:, :, :, D]
            nc.vector.memset(ones_cols, 1.0)
            for n in range(2):
                wvblk = wbig.tile([128, HC, 384], bf16, tag="wb", name="wv_blk")
                nc.sync.dma_start(wvblk[:], wv_d[l, n])
                for tt in range(TT):
                    pv = psA.tile([128, 384], f32, tag="main", name="pv")
                    for ic in range(HC):
                        nc.tensor.matmul(
                            pv[:], lhsT=xT[:, tt * H + ic * 128: tt * H + ic * 128 + 128],
                            rhs=wvblk[:, ic, :],
                            start=(ic == 0), stop=(ic == HC - 1))
                    # strided copy: 6 heads' 64-wide blocks into 65-wide slots
                    dst = V65t.rearrange("p (f h d) -> p f h d", f=TT, h=NH)[
                        :, tt, n * 6:(n + 1) * 6, 0:D]
                    src = pv.rearrange("p (h d) -> p h d", h=6)
                    nc.vector.tensor_copy(dst, src)

            if l == 0:
                dump("V65", V65t)
            # attention, head pairs
            attnT = tb.tile([128, HC * S], bf16, tag="t", name=f"attnT_{l}")
            v65v = V65t.rearrange("p (f h d) -> p f h d", f=TT, h=NH)
            for c in range(HC):
                es = [[None] * TT for _ in range(2)]
                for hh in range(2):
                    r0 = 64 * hh
                    for kc in range(TT):
                        sp = psA.tile([128, S], f32, tag="main", name="sp")
                        nc.tensor.matmul(
                            sp[:],
                            lhsT=KT[r0:r0 + 64, c * S + kc * 128: c * S + kc * 128 + 128],
                            rhs=QT[r0:r0 + 64, c * S:(c + 1) * S],
                            start=True, stop=True)
                        e = exps_p.tile([128, S], bf16, tag="e", name=f"e{hh}_{kc}")
                        mbias = mb_sb[:, kc:kc + 1] if use_mask else 0.0
                        nc.scalar.activation(e[:], sp[:], AF.Exp,
                                             bias=mbias, scale=0.125)
                        es[hh][kc] = e
                for hh in range(2):
                    h = 2 * c + hh
                    av = psV.tile([HD, S], f32, tag="av", name="av")
                    for kc in range(TT):
                        nc.tensor.matmul(
                            av[:], lhsT=v65v[:, kc, h, :], rhs=es[hh][kc][:],
                            start=(kc == 0), stop=(kc == TT - 1))
                    # regular-op ACT copy first: a custom-DVE read of a
                    # PSUM accumulation group races ahead of the group's tail
                    dn = rd_p.tile([1, S], f32, tag="dn", name="dn")
                    nc.scalar.copy(dn[:], av[D:HD, :])
                    rd = rd_p.tile([1, S], f32, tag="rd", name="rd")
                    nc.vector.reciprocal_approx_fast(out=rd[:], in_=dn[:])
                    if DBG and l == 0 and c == 0 and hh == 0:
                        for _kc in range(TT):
                            nc.sync.dma_start(dbg[f"es0{_kc}"][0:128, :],
                                              es[0][_kc][:])
                        avd = gb.tile([128, S], f32, tag="gb", name="avd")
                        nc.vector.tensor_copy(avd[0:HD, :], av[:])
                        nc.sync.dma_start(dbg["av0"][0:HD, :], avd[0:HD, :])
                        nc.sync.dma_start(dbg["rd0"][0:1, :], rd[:])
                    bcs = rd_p.tile([64, S], f32, tag="bcs", name="bcs")
                    if BCAST_MODE == "gpsimd":
                        nc.gpsimd.partition_broadcast(bcs[:], rd[0:1, :])
                    else:
                        rdb = rd_p.tile([1, S], bf16, tag="rdb", name="rdb")
                        nc.vector.tensor_copy(rdb[:], rd[:])
                        bcp = psB.tile([64, S], f32, tag="bc", name="bcp")
                        nc.tensor.matmul(bcp[:], lhsT=ones_bf[0:1, 0:64],
                                         rhs=rdb[:], start=True, stop=True)
                        nc.vector.tensor_copy(bcs[:], bcp[:])
                    if DBG and l == 0 and c == 0 and hh == 0:
                        nc.sync.dma_start(dbg["bcs0"][0:64, :], bcs[:])
                    dst = attnT[64 * hh:64 * hh + 64, c * S:(c + 1) * S]
                    nc.vector.tensor_tensor(out=dst, in0=av[0:D, :], in1=bcs[:],
                                            op=OP.mult)
                    if v_bias:
                        bv_sl = vec.tile([64, 1], f32, tag="bv", name="bv_sl")
                        nc.sync.dma_start(bv_sl[:], bv_d[l, h * D:(h + 1) * D][:, None])
                        nc.vector.tensor_scalar(
                            out=dst, in0=dst,
                            scalar1=bv_sl[:], scalar2=None, op0=OP.add)

            if l == 0:
                dump("attnT", attnT)
            _act_preload(nc, pools, AF.Sqrt)
            # Wo projection (+bo) + residual -> y (f32), LN1 -> y_bf (bf16)
            y = resf.tile([128, TT * H], f32, tag="res", name=f"y_{l}")
            y_bf = resbf.tile([128, TT * H], bf16, tag="rbf", name=f"ybf_{l}")
            g1_bc, b1_bc = _ln_bcast(nc, pools, g1_d[l], b1_d[l], ln1_aff)
            s1 = [stats.tile([128, TT], f32, tag=f"s{i}", name=f"s1_{l}{i}")
                  for i in range(2)]
            q1 = [stats.tile([128, TT], f32, tag=f"q{i}", name=f"q1_{l}{i}")
                  for i in range(2)]
            if o_bias:
                bo_row = brow_p.tile([1, H], f32r, tag="br", name="bo_row")
                nc.sync.dma_start(bo_row[:], bo_d[l][None, :])
            woblks = []
            for n in range(2):
                wob = wbig.tile([128, HC, 384], bf16, tag="wb", name=f"wo_blk{n}")
                nc.sync.dma_start(wob[:], wo_d[l, n])
                woblks.append(wob)
            for tt in range(TT):
                for n in range(2):
                    po = psA.tile([128, 384], f32, tag="main", name="po")
                    if o_bias:
                        nc.tensor.matmul(po[:], lhsT=ones_sb[0:1, 0:128],
                                         rhs=bo_row[0:1, n * 384:(n + 1) * 384],
                                         start=True, stop=False)
                    for jc in range(HC):
                        nc.tensor.matmul(
                            po[:],
                            lhsT=attnT[:, jc * S + tt * 128: jc * S + tt * 128 + 128],
                            rhs=woblks[n][:, jc, :],
                            start=(not o_bias and jc == 0), stop=(jc == HC - 1))
                    sl = slice(tt * H + n * 384, tt * H + n * 384 + 384)
                    nc.vector.scalar_tensor_tensor(
                        out=y[:, sl], in0=po[:, :], scalar=0.0,
                        in1=x_bf[:, sl], op0=OP.add, op1=OP.add,
                        accum_out=s1[n][:, tt:tt + 1])
                    _ln_stats_sq(nc, pools, y, tt, n, q1[n][:, tt:tt + 1])
                _ln_tile(nc, pools, y, y_bf, tt, s1[0][:, tt:tt + 1],
                         s1[1][:, tt:tt + 1], q1[0][:, tt:tt + 1],
                         q1[1][:, tt:tt + 1], g1_bc, b1_bc)

            if l == 0:
                dump("y", y)
                dump("y_bf", y_bf)
            # yT (DMA xbar), then preload the Gelu table while matmuls run
            yT = tb.tile([128, HC * S], bf16, tag="t", name=f"yT_{l}")
            for tt in range(TT):
                _dma_transpose_tile(nc, y_bf, yT, tt, pools)
            _act_preload(nc, pools, GELU_AF)
            if l == 0:
                dump("yT", yT)

            # FFN up: h1T = gelu(yT @ Wi + bi), hidden-major, bf16
            h1T = h1p.tile([128, FC * S], bf16, tag="h1", name=f"h1T_{l}")
            for fc in range(FC):
                wiblk = wsmall.tile([128, HC, 128], bf16, tag="ws", name="wi_blk")
                nc.sync.dma_start(wiblk[:], wi_d[l, fc])
                ph = psA.tile([128, S], f32, tag="main", name="ph")
                yTv = yT.rearrange("p (f c t) -> p f c t", f=TT, c=HC)
                for ic in range(HC):
                    nc.tensor.matmul(
                        ph[:], lhsT=wiblk[:, ic, :], rhs=yTv[:, :, ic, :],
                        start=(ic == 0), stop=(ic == HC - 1))
                if i_bias:
                    bi_sl = vec.tile([128, 1], f32, tag="v", name="bi_sl")
                    nc.sync.dma_start(bi_sl[:], bi_d[l, fc * 128:(fc + 1) * 128][:, None])
                    nc.scalar.activation(h1T[:, fc * S:(fc + 1) * S], ph[:], GELU_AF,
                                         bias=bi_sl[:])
                else:
                    nc.scalar.activation(h1T[:, fc * S:(fc + 1) * S], ph[:], GELU_AF)

            if l == 0:
                dump("h1T", h1T)
            # FFN down (bf16) + bd + residual -> ffnout; waves of 4 (tt,n) pairs
            ffnout = resf.tile([128, TT * H], f32, tag="res", name=f"ffnout_{l}")
            xbf_next = (None if last else
                        resbf.tile([128, TT * H], bf16, tag="rbf", name=f"xbf_{l + 1}"))
            g2_bc, b2_bc = _ln_bcast(nc, pools, g2_d[l], b2_d[l], ln2_aff)
            s2 = [stats.tile([128, TT], f32, tag=f"s{i}", name=f"s2_{l}{i}")
                  for i in range(2)]
            q2 = [stats.tile([128, TT], f32, tag=f"q{i}", name=f"q2_{l}{i}")
                  for i in range(2)]
            if d_bias:
                bd_row = brow_p.tile([1, H], f32r, tag="br", name="bd_row")
                nc.sync.dma_start(bd_row[:], bd_d[l][None, :])
            for wave in range(2):
                tts = (0, 1) if wave == 0 else (2, 3)
                wave_pairs = [(tt, n) for tt in tts for n in range(2)]
                accs = {}
                for (tt, n) in wave_pairs:
                    acc = psA.tile([128, 384], f32, tag="main", name=f"acc{tt}_{n}")
                    if d_bias:
                        nc.tensor.matmul(acc[:], lhsT=ones_sb[0:1, 0:128],
                                         rhs=bd_row[0:1, n * 384:(n + 1) * 384],
                                         start=True, stop=False)
                    accs[(tt, n)] = acc
                for fp in range(FC // 4):
                    wdblk = wdp.tile([128, 4, H], bf16, tag="wd", name="wd_blk")
                    nc.sync.dma_start(wdblk[:], wd_d[l, fp])
                    for two in range(4):
                        fc = 4 * fp + two
                        for (tt, n) in wave_pairs:
                            nc.tensor.matmul(
                                accs[(tt, n)][:],
                                lhsT=h1T[:, fc * S + tt * 128: fc * S + tt * 128 + 128],
                                rhs=wdblk[:, two, n * 384:(n + 1) * 384],
                                start=(not d_bias and fc == 0), stop=(fc == FC - 1))
                if wave == 0:
                    # sqrt table back in residence before LN2's rstd
                    _act_preload(nc, pools, AF.Sqrt)
                for tt in tts:
                    for n in range(2):
                        sl = slice(tt * H + n * 384, tt * H + n * 384 + 384)
                        nc.vector.scalar_tensor_tensor(
                            out=ffnout[:, sl], in0=accs[(tt, n)][:, :],
                            scalar=0.0, in1=y_bf[:, sl], op0=OP.add,
                            op1=OP.add, accum_out=s2[n][:, tt:tt + 1])
                        _ln_stats_sq(nc, pools, ffnout, tt, n,
                                     q2[n][:, tt:tt + 1])
                    if last:
                        x_out = pools.get("x_out")
                        if x_out is None:
                            x_out = scratch.tile([128, TT * H], f32, tag="xo",
                                                 name="x_out")
                            pools["x_out"] = x_out
                        _ln_tile(nc, pools, ffnout, x_out, tt,
                                 s2[0][:, tt:tt + 1], s2[1][:, tt:tt + 1],
                                 q2[0][:, tt:tt + 1], q2[1][:, tt:tt + 1],
                                 g2_bc, b2_bc)
                        nc.sync.dma_start(
                            out_d[tt * 128:(tt + 1) * 128, :],
                            x_out[:, tt * H:(tt + 1) * H])
                    else:
                        _ln_tile(nc, pools, ffnout, xbf_next, tt,
                                 s2[0][:, tt:tt + 1], s2[1][:, tt:tt + 1],
                                 q2[0][:, tt:tt + 1], q2[1][:, tt:tt + 1],
                                 g2_bc, b2_bc)
            if l == 0:
                dump("ffnout", ffnout)
            x_bf = xbf_next

    nc.compile()
    return nc


def _prep_inputs(inputs, b):
    f = np.float32
    bh = ml_dtypes.bfloat16
    Wq, Wk, Wv, Wo, Wi = (np.asarray(inputs[k], f) for k in ("Wq", "Wk", "Wv", "Wo", "Wi"))
    WqS = np.ascontiguousarray(
        Wq.reshape(L, HC, 128, HC, 128).transpose(0, 3, 2, 1, 4)).astype(bh)
    WkS = np.ascontiguousarray(
        Wk.reshape(L, HC, 128, HC, 128).transpose(0, 3, 2, 1, 4)).astype(bh)
    WvS = np.ascontiguousarray(
        Wv.reshape(L, HC, 128, 2, 384).transpose(0, 3, 2, 1, 4)).astype(bh)
    WoS = np.ascontiguousarray(
        Wo.reshape(L, HC, 128, 2, 384).transpose(0, 3, 2, 1, 4)).astype(bh)
    WiS = np.ascontiguousarray(
        Wi.reshape(L, HC, 128, FC, 128).transpose(0, 3, 2, 1, 4)).astype(bh)
    Wd = np.asarray(inputs["Wd"], f)
    WdB = np.ascontiguousarray(
        Wd.reshape(L, FC // 4, 4, 128, H).transpose(0, 1, 3, 2, 4)).astype(bh)
    mask = np.asarray(inputs["input_mask"], f)
    tti = np.asarray(inputs["token_type_ids"], np.int32)
    flags = dict(
        qk_bias=bool(np.any(np.asarray(inputs["bq"])) or np.any(np.asarray(inputs["bk"]))),
        v_bias=bool(np.any(np.asarray(inputs["bv"]))),
        o_bias=bool(np.any(np.asarray(inputs["bo"]))),
        i_bias=bool(np.any(np.asarray(inputs["bi"]))),
        d_bias=bool(np.any(np.asarray(inputs["bd"]))),
        ln1_aff=bool(np.any(np.asarray(inputs["ln1_g"]) != 1.0) or
                     np.any(np.asarray(inputs["ln1_b"]))),
        ln2_aff=bool(np.any(np.asarray(inputs["ln2_g"]) != 1.0) or
                     np.any(np.asarray(inputs["ln2_b"]))),
        emb_aff=bool(np.any(np.asarray(inputs["emb_ln_g"]) != 1.0) or
                     np.any(np.asarray(inputs["emb_ln_b"]))),
        use_mask=bool(np.any(mask != 1.0)),
        use_type=bool(np.any(tti != 0)),
    )
    pos_eff = np.asarray(inputs["pos_emb"], f)[:S]
    if not flags["use_type"]:
        pos_eff = pos_eff + np.asarray(inputs["type_emb"], f)[int(tti.flat[0])][None, :]
    shared = dict(
        tok_emb=np.asarray(inputs["tok_emb"], f),
        pos_emb=pos_eff,
        type_emb=np.asarray(inputs["type_emb"], f),
        emb_g=np.asarray(inputs["emb_ln_g"], f),
        emb_b=np.asarray(inputs["emb_ln_b"], f),
        WqS=WqS, WkS=WkS, WvS=WvS, WoS=WoS, WiS=WiS, WdB=WdB,
        bq=np.asarray(inputs["bq"], f), bk=np.asarray(inputs["bk"], f),
        bv=np.asarray(inputs["bv"], f), bo=np.asarray(inputs["bo"], f),
        bi=np.asarray(inputs["bi"], f), bd=np.asarray(inputs["bd"], f),
        ln1_g=np.asarray(inputs["ln1_g"], f), ln1_b=np.asarray(inputs["ln1_b"], f),
        ln2_g=np.asarray(inputs["ln2_g"], f), ln2_b=np.asarray(inputs["ln2_b"], f),
        ones=np.ones((128, 128), f),
        ident=np.eye(128, dtype=ml_dtypes.bfloat16),
    )
    in_maps = []
    ids = np.asarray(inputs["input_ids"], np.int32)
    for c in range(b):
        m = dict(shared)
        m["ids"] = np.ascontiguousarray(ids[c])
        m["tti"] = np.ascontiguousarray(tti[c])
        m["mb"] = np.ascontiguousarray((1.0 - mask[c]) * -10000.0)
        in_maps.append(m)
    return in_maps, flags


def kernel(**inputs):
    global LAST_EXEC_TIME_NS
    n_layers = int(os.environ.get("BERT_LAYERS", L))
    trace = bool(os.environ.get("BERT_TRACE"))
    in_maps, flags = _prep_inputs(inputs, B)
    nc = build(n_layers, flags)
    res = bass_utils.run_bass_kernel_spmd(
        nc, in_maps, core_ids=list(range(B)), trace=trace)
    LAST_EXEC_TIME_NS = res.exec_time_ns
    out = np.stack([res.results[c]["out"] for c in range(B)])
    return out.astype(np.float32)
